# revision 2
# baseline (speedup 1.0000x reference)
"""Trainium2 Bass kernel for nn_CANLayer (CAN GNN layer) — v2.

Strategy (8-core SPMD, no collectives):
  - Targets sharded 6272/core (49 tiles of 128, 128-aligned ownership).
    Edges routed to the target-owning core; every softmax segment is local.
  - Phase 1 (scores): node-major matvec on PE (lhsT = x^T blocks, rhs = the
    four projected attention vectors) -> sc_sb [128, 392, 4]; ss pair is
    written into the combined gather table xs_tab cols 256:260 (bf16-packed
    f32), st pair for own targets into st_tab.  No transposes, 3 DMAs.
  - Combined gather row (768B): [x bf16(256) | ss_l,ss_u f32(8B) | pad].
    One dma_gather per edge fetches x AND ss.  st gathered separately from
    st_tab (256B rows) with tile-local int16 indices.
  - Per 128-edge chunk: fused one-hot build maskw[e,t] = (iota==tof)*w, then
    U^T[d,t] += Xg^T-form matmuls (lhsT = xg d-halves, rhs = maskw) and
    s[t] += maskw^T @ 1 (lhsT = maskw, rhs = ones).  A^T (unnormalized U^T)
    stays SBUF-resident in bf16; 1/s kept per tile.
  - Phase 3: out = relu(rs_l*(U_l@W_l) + rs_u*(U_u@W_u) + x@(W_lin*EPS)),
    lhsT = resident A^T / xot slices; per-partition rs scaling; batched
    output writes.  Whole-conv metadata loaded in a few mega-DMAs.
"""

import os
import sys
from contextlib import ExitStack

import numpy as np
import ml_dtypes

for _p in ("/opt/trn_rl_repo", "/root/.axon_site/_ro/trn_rl_repo"):
    if os.path.isdir(_p) and _p not in sys.path:
        sys.path.insert(0, _p)

import concourse.bass as bass
import concourse.bacc as bacc
import concourse.tile as tile
from concourse import mybir
from concourse.bass_utils import run_bass_kernel_spmd

BF16 = mybir.dt.bfloat16
FP8 = mybir.dt.float8e4
F32 = mybir.dt.float32
I16 = mybir.dt.int16
ALU = mybir.AluOpType
ACTF = mybir.ActivationFunctionType

EPS = 1.0 + 1e-06
PAD_OFF = 200.0  # tgt_off value for pad edge slots (outside [0,128) window)
ROW = 384        # xs_tab row cols (bf16) = 768 bytes
SS0 = 256        # first score col in xs_tab


def _ceil(a, b):
    return -(-a // b)


# --------------------------------------------------------------------------
# host-side preprocessing
# --------------------------------------------------------------------------

def _prep_conv(indices, values, n, n_cores, own, ntiles, split):
    """Sort/tile/pad one conv's edges.  Returns per-core arrays + baked meta.

    Per tile (128 consecutive owned targets): edges are grouped as
    [A-edges (src < split) | pad | B-edges | pad], each padded up to a
    multiple of 128 with (src=0, val=0, tgt_off=PAD_OFF).  Chunk counts are
    maxed over cores so one SPMD program fits all cores.
    """
    tgt = np.asarray(indices[0], dtype=np.int64).astype(np.int32)
    src = np.asarray(indices[1], dtype=np.int64).astype(np.int32)
    val = np.asarray(values, dtype=np.float32)

    core_of = np.minimum(tgt // own, n_cores - 1)

    per_core = []
    for k in range(n_cores):
        sel = np.nonzero(core_of == k)[0]
        tl = tgt[sel] - k * own
        order = np.argsort(tl, kind="stable")
        sel = sel[order]
        tl = tl[order]
        tile_id = tl >> 7
        bounds = np.searchsorted(tile_id, np.arange(ntiles + 1))
        tiles = []
        for t in range(ntiles):
            e = sel[bounds[t]:bounds[t + 1]]
            isa = src[e] < split
            tiles.append((e[isa], e[~isa]))
        per_core.append(tiles)

    ncha = [max(_ceil(max(len(per_core[k][t][0]) for k in range(n_cores)), 128), 1)
            for t in range(ntiles)]
    nchb = [_ceil(max(len(per_core[k][t][1]) for k in range(n_cores)), 128)
            for t in range(ntiles)]
    ch = [a + b for a, b in zip(ncha, nchb)]
    cht = sum(ch)
    fa = sum(ncha) * 8   # int16 idx cols (128 idx -> 8 cols of 16)
    fb = sum(nchb) * 8

    def wrap_idx(vals16, out, col0):
        # linear idx i -> (partition i%16 [+16*g replicas], col i//16)
        m = len(vals16) // 16
        blk = vals16.reshape(m, 16).T  # [16, m]
        for g in range(8):
            out[g * 16:(g + 1) * 16, col0:col0 + m] = blk

    cores = []
    for k in range(n_cores):
        idxa = np.zeros((128, fa), np.int16)
        idxb = np.zeros((128, max(fb, 1)), np.int16)
        toff = np.full((128, cht), PAD_OFF, np.float32)
        vals = np.zeros((128, cht), np.float32)
        toffi = np.zeros((128, cht * 8), np.int16)
        ca = cb = cc = 0
        for t in range(ntiles):
            ea, eb = per_core[k][t]
            for which, e, nch in (("a", ea, ncha[t]), ("b", eb, nchb[t])):
                nslot = nch * 128
                s = np.zeros(nslot, np.int32)
                s[:len(e)] = src[e] if which == "a" else src[e] - split
                to = np.full(nslot, PAD_OFF, np.float32)
                to[:len(e)] = (tgt[e] - k * own - t * 128).astype(np.float32)
                vv = np.zeros(nslot, np.float32)
                vv[:len(e)] = val[e]
                # chunk-major [p, c] layout: slot i -> (i % 128, i // 128)
                cols = slice(cc, cc + nch)
                toff[:, cols] = to.reshape(nch, 128).T
                vals[:, cols] = vv.reshape(nch, 128).T
                ti = np.zeros(nslot, np.int32)
                ti[:len(e)] = (tgt[e] - k * own - t * 128).astype(np.int32)
                wrap_idx(ti.astype(np.int16), toffi, cc * 8)
                cc += nch
                if which == "a":
                    wrap_idx(s.astype(np.int16), idxa, ca * 8)
                    ca += nch
                else:
                    if nch:
                        wrap_idx(s.astype(np.int16), idxb, cb * 8)
                    cb += nch
        cores.append(dict(idxa=idxa, idxb=idxb, toff=toff, val=vals,
                          toffi=toffi))

    meta = dict(ncha=ncha, nchb=nchb, ch=ch, cht=cht, fa=fa, fb=max(fb, 1),
                ownp=ntiles * 128)
    return meta, cores


# --------------------------------------------------------------------------
# device program
# --------------------------------------------------------------------------

def psum2_tile(p, tag):
    # one full PSUM bank (2 KB/partition) so each matmul accumulation group
    # owns its zero region exclusively
    return p.tile([128, 512], F32, tag=tag, name=f"pb_{tag}")


def _build_program(n, npad, d, n_cores, own, ntiles, split, meta_l, meta_u):
    nc = bacc.Bacc(trn_type="TRN2", target_bir_lowering=False, debug=False,
                   num_devices=n_cores, num_swdge_queues=4)
    ownp = ntiles * 128
    nblk = npad // 128          # 128-node score blocks
    J = 1792                    # phase-1 xt chunk cols (14 blocks)
    assert npad % J == 0
    niter = npad // J

    def din(name, shape, dt):
        return nc.dram_tensor(name, shape, dt, kind="ExternalInput")

    xs_tab = din("xs_tab", [npad, ROW], BF16)
    xt2 = din("xt2", [128, 2, npad], BF16)
    xot2 = din("xot2", [128, 2, ownp], BF16)
    wa42 = din("wa42", [128, 2, 4], BF16)
    w_l2 = din("w_l2", [128, 2, d], BF16)
    w_u2 = din("w_u2", [128, 2, d], BF16)
    w_lin2 = din("w_lin2", [128, 2, d], BF16)
    iota_in = din("iota_in", [128, 128], BF16)
    convs = {}
    for cv, meta in (("l", meta_l), ("u", meta_u)):
        convs[cv] = dict(
            meta=meta,
            idxa=din(f"idxa_{cv}", [128, meta["fa"]], I16),
            idxb=din(f"idxb_{cv}", [128, meta["fb"]], I16),
            toff=din(f"toff_{cv}", [128, meta["cht"]], F32),
            val=din(f"val_{cv}", [128, meta["cht"]], F32),
            toffi=din(f"toffi_{cv}", [128, meta["cht"] * 8], I16),
        )
    st_tab = nc.dram_tensor("st_tab", [ownp, 64], F32)
    out = nc.dram_tensor("out", [ownp, d], F32, kind="ExternalOutput")

    gmax = 8   # max chunks (x128 idx) per dma_gather call (SWDGE ring)

    def split_gather(out_tile, co, nch, table, idx_sb, io, elem, qiter):
        g0 = 0
        while g0 < nch:
            g = min(gmax, nch - g0)
            nc.gpsimd.dma_gather(
                out_tile[:, co + g0:co + g0 + g, :], table,
                idx_sb[:, (io + g0) * 8:(io + g0 + g) * 8],
                g * 128, g * 128, elem, elem_step=elem, queue_num=qiter[0] % 4)
            qiter[0] += 1
            g0 += g

    with tile.TileContext(nc) as tc:
        with ExitStack() as ctx:
            pool = ctx.enter_context(tc.tile_pool(name="sb", bufs=2))
            mpool = ctx.enter_context(tc.tile_pool(name="meta", bufs=1))
            cpool = ctx.enter_context(tc.tile_pool(name="const", bufs=1))

            iota_t = cpool.tile([128, 128], BF16)
            nc.sync.dma_start(iota_t[:], iota_in[:, :])
            ones_t = cpool.tile([128, 1], BF16)
            nc.vector.memset(ones_t[:], 1.0)
            wa_sb = cpool.tile([128, 2, 4], BF16)
            nc.sync.dma_start(wa_sb[:], wa42[:, :, :])
            xo_sb = cpool.tile([128, 2, ownp], BF16)
            nc.sync.dma_start(xo_sb[:], xot2[:, :, :])

            # ---- phase 1: scores, node-major on PE -----------------------
            p1cm = tc.tile_pool(name="p1", bufs=2)
            p1 = p1cm.__enter__()
            sc_sb = p1.tile([128, nblk, 4], F32, tag="scsb", bufs=1)
            with tc.tile_pool(name="ps1", bufs=2, space="PSUM") as ps1:
                for it in range(niter):
                    xt_t = p1.tile([128, 2, J], BF16, tag="xt")
                    nc.sync.dma_start(xt_t[:], xt2[:, :, it * J:(it + 1) * J])
                    scp = ps1.tile([128, J // 128, 4], F32, tag="scp", bufs=2)
                    for b in range(J // 128):
                        for h in range(2):
                            nc.tensor.matmul(
                                out=scp[:, b, :],
                                lhsT=xt_t[:, h, b * 128:(b + 1) * 128],
                                rhs=wa_sb[:, h, :],
                                start=(h == 0), stop=(h == 1))
                    nc.vector.tensor_copy(
                        out=sc_sb[:, it * (J // 128):(it + 1) * (J // 128), :],
                        in_=scp[:])
                # ss pair -> xs_tab cols [SS0, SS0+4) as raw f32-pair bytes
                nc.sync.dma_start(
                    xs_tab[0:npad, SS0:SS0 + 4].rearrange(
                        "(c p) f -> p c f", p=128),
                    sc_sb[:, :, 0:2].bitcast(BF16))

                # own-target st pair from resident xot2
                stp = ps1.tile([128, ntiles, 2], F32, tag="stp", bufs=1)
                for b in range(ntiles):
                    for h in range(2):
                        nc.tensor.matmul(
                            out=stp[:, b, :],
                            lhsT=xo_sb[:, h, b * 128:(b + 1) * 128],
                            rhs=wa_sb[:, h, 2:4],
                            start=(h == 0), stop=(h == 1))
                sto_sb = p1.tile([128, ntiles, 2], F32, tag="sto")
                nc.vector.tensor_copy(out=sto_sb[:], in_=stp[:])
                nc.sync.dma_start(
                    st_tab[0:ownp, 0:2].rearrange("(c p) f -> p c f", p=128),
                    sto_sb[:])

            wsb = {}
            for nm, w_in in (("l", w_l2), ("u", w_u2), ("x", w_lin2)):
                tl = cpool.tile([128, 2, d], BF16, tag=f"w{nm}",
                                name=f"w_{nm}")
                nc.sync.dma_start(tl[:], w_in[:, :, :])
                wsb[nm] = tl

            p1cm.__exit__(None, None, None)
            tc.strict_bb_all_engine_barrier()

            # per-conv metadata mega-loads
            msb = {}
            for cv in ("l", "u"):
                cd = convs[cv]
                meta = cd["meta"]
                d_ = {}
                d_["ia"] = mpool.tile([128, meta["fa"]], I16, tag=f"ia{cv}",
                                      name=f"ia_{cv}")
                nc.sync.dma_start(d_["ia"][:], cd["idxa"][:, :])
                d_["ib"] = mpool.tile([128, meta["fb"]], I16, tag=f"ib{cv}",
                                      name=f"ib_{cv}")
                nc.sync.dma_start(d_["ib"][:], cd["idxb"][:, :])
                d_["ti"] = mpool.tile([128, meta["cht"] * 8], I16,
                                      tag=f"ti{cv}", name=f"ti_{cv}")
                nc.sync.dma_start(d_["ti"][:], cd["toffi"][:, :])
                d_["tof"] = mpool.tile([128, meta["cht"]], F32, tag=f"tof{cv}",
                                       name=f"tof_{cv}")
                nc.sync.dma_start(d_["tof"][:], cd["toff"][:, :])
                d_["val"] = mpool.tile([128, meta["cht"]], F32, tag=f"val{cv}",
                                       name=f"val_{cv}")
                nc.sync.dma_start(d_["val"][:], cd["val"][:, :])
                msb[cv] = d_
            # ---- phase 2+3 interleaved: per tile, conv l then u,
            # then immediate projection + combine + relu ------------------
            qiter = [0]
            ps2 = ctx.enter_context(tc.tile_pool(name="ps2", bufs=1,
                                                 space="PSUM"))
            ps3 = ctx.enter_context(tc.tile_pool(name="ps3", bufs=2,
                                                 space="PSUM"))
            p3 = ctx.enter_context(tc.tile_pool(name="p3", bufs=2))
            offs = {}
            for cv in ("l", "u"):
                meta = convs[cv]["meta"]
                offs[cv] = (np.cumsum([0] + meta["ncha"]),
                            np.cumsum([0] + meta["nchb"]),
                            np.cumsum([0] + meta["ch"]))
            OB = 7  # tiles per output write batch (49 = 7*7)
            ost = None
            for t in range(ntiles):
                uts = {}
                rss = {}
                for cvi, cv in enumerate(("l", "u")):
                    meta = convs[cv]["meta"]
                    na, nb = meta["ncha"][t], meta["nchb"][t]
                    ch = meta["ch"][t]
                    offa, offb, offc = offs[cv]
                    c0 = offc[t]
                    ia_sb, ib_sb = msb[cv]["ia"], msb[cv]["ib"]
                    ti_sb = msb[cv]["ti"]
                    tof_sb, val_sb = msb[cv]["tof"], msb[cv]["val"]

                    xg = pool.tile([128, ch, ROW], BF16, tag=f"xg{cv}",
                                   name=f"xg_{cv}")
                    split_gather(xg, 0, na, xs_tab[:, :], ia_sb, offa[t],
                                 ROW, qiter)
                    if nb:
                        split_gather(xg, na, nb, xs_tab[split:, :], ib_sb,
                                     offb[t], ROW, qiter)
                    stg = pool.tile([128, ch, 64], F32, tag=f"stg{cv}",
                                    name=f"stg_{cv}")
                    split_gather(stg, 0, ch, st_tab[t * 128:, :], ti_sb,
                                 c0, 64, qiter)

                    ss = xg[:, :, SS0 + 2 * cvi:SS0 + 2 * cvi + 2].bitcast(F32)
                    st = stg[:, :, cvi:cvi + 1]
                    tof = tof_sb[:, c0:c0 + ch]
                    vv = val_sb[:, c0:c0 + ch]

                    # w = exp(elu(st+ss) * val); elu(z)=max(z,0)-1+e^min(z,0)
                    z = pool.tile([128, ch], F32, tag=f"z{cv}",
                                  name=f"z_{cv}")
                    nc.vector.tensor_add(out=z[:], in0=st, in1=ss)
                    zm = pool.tile([128, ch], F32, tag=f"zm{cv}",
                                   name=f"zm_{cv}")
                    nc.vector.tensor_scalar(out=zm[:], in0=z[:], scalar1=0.0,
                                            scalar2=None, op0=ALU.min)
                    e1 = pool.tile([128, ch], F32, tag=f"e1{cv}",
                                   name=f"e1_{cv}")
                    nc.scalar.activation(e1[:], zm[:], ACTF.Exp)
                    zp = pool.tile([128, ch], F32, tag=f"zp{cv}",
                                   name=f"zp_{cv}")
                    nc.vector.tensor_scalar(out=zp[:], in0=z[:], scalar1=0.0,
                                            scalar2=-1.0, op0=ALU.max,
                                            op1=ALU.add)
                    t1 = pool.tile([128, ch], F32, tag=f"t1{cv}",
                                   name=f"t1_{cv}")
                    nc.vector.tensor_add(out=t1[:], in0=zp[:], in1=e1[:])
                    v = pool.tile([128, ch], F32, tag=f"v{cv}",
                                  name=f"v_{cv}")
                    nc.vector.tensor_mul(out=v[:], in0=t1[:], in1=vv)
                    w = pool.tile([128, ch], F32, tag=f"w{cv}",
                                  name=f"w_{cv}")
                    nc.scalar.activation(w[:], v[:], ACTF.Exp)

                    ut0 = psum2_tile(ps2, "ut0")
                    ut1 = psum2_tile(ps2, "ut1")
                    s_ps = psum2_tile(ps2, "sps")
                    for c in range(ch):
                        mw = pool.tile([128, 128], BF16, tag=f"mw{cv}",
                                       name=f"mw_{cv}", bufs=8)
                        nc.vector.tensor_scalar(
                            out=mw[:], in0=iota_t[:], scalar1=tof[:, c:c + 1],
                            scalar2=w[:, c:c + 1], op0=ALU.is_equal,
                            op1=ALU.mult)
                        nc.tensor.matmul(
                            out=ut0[:, 0:128], lhsT=xg[:, c, 0:128], rhs=mw[:],
                            start=(c == 0), stop=(c == ch - 1))
                        nc.tensor.matmul(
                            out=ut1[:, 0:128], lhsT=xg[:, c, 128:256],
                            rhs=mw[:],
                            start=(c == 0), stop=(c == ch - 1))
                        nc.tensor.matmul(out=s_ps[:, 0:1], lhsT=mw[:],
                                         rhs=ones_t[:],
                                         start=(c == 0), stop=(c == ch - 1))
                    atk = pool.tile([128, 2, 128], BF16, tag=f"at{cv}",
                                    name=f"at_{cv}")
                    nc.vector.tensor_copy(out=atk[:, 0, :], in_=ut0[:, 0:128])
                    nc.vector.tensor_copy(out=atk[:, 1, :], in_=ut1[:, 0:128])
                    uts[cv] = atk
                    sden = pool.tile([128, 1], F32, tag=f"sden{cv}",
                                     name=f"sden_{cv}")
                    nc.vector.tensor_scalar(out=sden[:], in0=s_ps[:, 0:1],
                                            scalar1=1e-30, scalar2=None,
                                            op0=ALU.max)
                    rsk = pool.tile([128, 1], F32, tag=f"rs{cv}",
                                    name=f"rs_{cv}")
                    nc.vector.reciprocal(rsk[:], sden[:])
                    rss[cv] = rsk

                # ---- phase 3 for tile t --------------------------------
                i = t % OB
                if i == 0:
                    ost = p3.tile([128, OB, d], F32, tag="ost")
                tc0, tc1 = t * 128, (t + 1) * 128
                o123 = ps3.tile([128, 3, d], F32, tag="o123")
                for j, (atk, wk) in enumerate(
                        ((uts["l"], "l"), (uts["u"], "u"), (None, "x"))):
                    for h in range(2):
                        lhs = (atk[:, h, :] if atk is not None
                               else xo_sb[:, h, tc0:tc1])
                        nc.tensor.matmul(out=o123[:, j, :], lhsT=lhs,
                                         rhs=wsb[wk][:, h, :],
                                         start=(h == 0), stop=(h == 1))
                a1 = p3.tile([128, d], F32, tag="a1")
                nc.vector.tensor_scalar(out=a1[:], in0=o123[:, 0, :],
                                        scalar1=rss["l"][:], scalar2=None,
                                        op0=ALU.mult)
                a2 = p3.tile([128, d], F32, tag="a2")
                nc.vector.tensor_scalar(out=a2[:], in0=o123[:, 1, :],
                                        scalar1=rss["u"][:], scalar2=None,
                                        op0=ALU.mult)
                a3 = p3.tile([128, d], F32, tag="a1")
                nc.vector.tensor_add(out=a3[:], in0=a1[:], in1=a2[:])
                a4 = p3.tile([128, d], F32, tag="a2")
                nc.vector.tensor_add(out=a4[:], in0=a3[:], in1=o123[:, 2, :])
                nc.scalar.activation(ost[:, i, :], a4[:], ACTF.Relu)
                if i == OB - 1 or t == ntiles - 1:
                    t0 = t - i
                    nc.sync.dma_start(
                        out[t0 * 128:(t + 1) * 128, :].rearrange(
                            "(b p) f -> p b f", p=128),
                        ost[:, 0:i + 1, :])
    import re as _re
    for blk in nc.m.functions[0].blocks:
        for inst in blk.instructions:
            if isinstance(inst, mybir.InstDMAGatherAnt):
                lane = None
                si = inst.sync_info
                ups = si.on_update if si is not None else []
                for u in ups:
                    m = _re.search(r"DMASW(\d+)", str(u.ant_name))
                    if m:
                        lane = int(m.group(1))
                        break
                if lane is not None:
                    inst.queue_num = lane % 4
    nc.finalize()
    return nc


# --------------------------------------------------------------------------
# top level
# --------------------------------------------------------------------------

def _prepare(x, lower_indices, lower_values, upper_indices, upper_values,
             W_lower, att_lower, W_upper, att_upper, W_lin,
             n_cores=8, split=32768):
    n, d = x.shape
    ntiles = _ceil(_ceil(n, n_cores), 128)
    own = ntiles * 128          # 128-aligned ownership (6272)
    ownp = own
    npad = _ceil(max(n_cores * ownp, n), 512) * 512

    meta_l, cores_l = _prep_conv(lower_indices, lower_values, n, n_cores, own,
                                 ntiles, split)
    meta_u, cores_u = _prep_conv(upper_indices, upper_values, n, n_cores, own,
                                 ntiles, split)

    xf = np.asarray(x, np.float32)
    x_pad = np.zeros((npad, d), np.float32)
    x_pad[:n] = xf
    xs_np = np.zeros((npad, ROW), ml_dtypes.bfloat16)
    xs_np[:, :d] = x_pad.astype(ml_dtypes.bfloat16)
    xt_b = np.ascontiguousarray(x_pad.T).astype(ml_dtypes.bfloat16)  # [d, npad]
    xt2 = np.ascontiguousarray(
        xt_b.reshape(2, 128, npad).transpose(1, 0, 2))  # [128, 2, npad]

    wl = np.asarray(W_lower, np.float32)
    wu = np.asarray(W_upper, np.float32)
    wlin = np.asarray(W_lin, np.float32) * EPS
    al = np.asarray(att_lower, np.float32)
    au = np.asarray(att_upper, np.float32)
    # wa4 cols -> fields: [ss_l, ss_u, st_l, st_u]
    wa4 = np.stack([wl @ al[d:], wu @ au[d:], wl @ al[:d], wu @ au[:d]],
                   axis=1).astype(ml_dtypes.bfloat16)   # [d, 4]
    wa42 = np.ascontiguousarray(wa4.reshape(2, 128, 4).transpose(1, 0, 2))

    def w2(w):
        return np.ascontiguousarray(
            w.astype(ml_dtypes.bfloat16).reshape(2, 128, d).transpose(1, 0, 2))

    iota_np = np.broadcast_to(np.arange(128), (128, 128)).astype(
        ml_dtypes.bfloat16)

    common = dict(xs_tab=xs_np, xt2=xt2, wa42=wa42,
                  w_l2=w2(wl), w_u2=w2(wu), w_lin2=w2(wlin),
                  iota_in=iota_np)
    in_maps = []
    for k in range(n_cores):
        m = dict(common)
        xo = np.ascontiguousarray(x_pad[k * own:k * own + ownp].T).astype(
            ml_dtypes.bfloat16)
        m["xot2"] = np.ascontiguousarray(
            xo.reshape(2, 128, ownp).transpose(1, 0, 2))
        for cv, cores in (("l", cores_l), ("u", cores_u)):
            cdk = cores[k]
            m[f"idxa_{cv}"] = cdk["idxa"]
            m[f"idxb_{cv}"] = cdk["idxb"]
            m[f"toff_{cv}"] = cdk["toff"]
            m[f"val_{cv}"] = cdk["val"]
            m[f"toffi_{cv}"] = cdk["toffi"]
        in_maps.append(m)

    dims = dict(n=n, npad=npad, d=d, n_cores=n_cores, own=own, ntiles=ntiles,
                split=split)
    return dims, meta_l, meta_u, in_maps


def build_all(inputs, n_cores=8, split=32768):
    dims, meta_l, meta_u, in_maps = _prepare(**inputs, n_cores=n_cores,
                                             split=split)
    nc = _build_program(dims["n"], dims["npad"], dims["d"], dims["n_cores"],
                        dims["own"], dims["ntiles"], dims["split"],
                        meta_l, meta_u)
    return nc, in_maps, dims


def kernel(**inputs):
    nc, in_maps, dims = build_all(inputs)
    res = run_bass_kernel_spmd(nc, in_maps, list(range(dims["n_cores"])))
    outs = [res.results[k]["out"][:dims["own"]] for k in range(dims["n_cores"])]
    return np.concatenate(outs, axis=0)[:dims["n"]].astype(np.float32)


# revision 3
# speedup vs baseline: 1.0625x; 1.0625x over previous
"""Trainium2 Bass kernel for nn_CANLayer (CAN GNN layer) — v2.

Strategy (8-core SPMD, no collectives):
  - Targets sharded 6272/core (49 tiles of 128, 128-aligned ownership).
    Edges routed to the target-owning core; every softmax segment is local.
  - Phase 1 (scores): node-major matvec on PE (lhsT = x^T blocks, rhs = the
    four projected attention vectors); the ss pair is written into the
    combined gather table xs_tab cols 256:260 (f32 pair packed in bf16
    cols) in one descriptor-heavy DMA, the st pair for own targets into
    st_tab.  No transposes, no zeroing passes.
  - Combined gather row (768B): [x bf16(256) | ss_l,ss_u f32(8B) | pad].
    One dma_gather per 128-edge chunk group fetches x AND ss (<=1024
    indices per call: HW SWDGE ring cap).  st gathered separately from
    st_tab (256B rows) with tile-local int16 indices.
  - Per 128-edge chunk: fused one-hot build maskw[e,t] = (iota==tof)*w with
    w = exp(elu(st+ss)*val) computed in 6 batched ops; then
    U^T[d,t] += matmul(lhsT=xg d-half, rhs=maskw) into per-bank PSUM and
    s[t] += matmul(lhsT=maskw, rhs=ones) (s as a column for per-partition
    scaling later).  Unnormalized U^T copied to bf16 per tile.
  - Convs l/u interleaved per tile; phase 3 fused into the tile loop:
    out = relu(rs_l*(U_l^T.T@W_l) + rs_u*(U_u^T.T@W_u) + x@(W_lin*EPS))
    with rs = 1/max(s,eps) per-partition scalars, batched 7-tile output
    writes.  Whole-conv idx/toff/val metadata loaded in a few mega-DMAs.
"""

import os
import sys
from contextlib import ExitStack

import numpy as np
import ml_dtypes

for _p in ("/opt/trn_rl_repo", "/root/.axon_site/_ro/trn_rl_repo"):
    if os.path.isdir(_p) and _p not in sys.path:
        sys.path.insert(0, _p)

import concourse.bass as bass
import concourse.bacc as bacc
import concourse.tile as tile
from concourse import mybir
from concourse.bass_utils import run_bass_kernel_spmd

BF16 = mybir.dt.bfloat16
FP8 = mybir.dt.float8e4
F32 = mybir.dt.float32
I16 = mybir.dt.int16
ALU = mybir.AluOpType
ACTF = mybir.ActivationFunctionType

EPS = 1.0 + 1e-06
PAD_OFF = 200.0  # tgt_off value for pad edge slots (outside [0,128) window)
ROW = 384        # xs_tab row cols (bf16) = 768 bytes
SS0 = 256        # first score col in xs_tab


def _ceil(a, b):
    return -(-a // b)


# --------------------------------------------------------------------------
# host-side preprocessing
# --------------------------------------------------------------------------

def _prep_conv(indices, values, n, n_cores, own, ntiles, split):
    """Sort/tile/pad one conv's edges.  Returns per-core arrays + baked meta.

    Per tile (128 consecutive owned targets): edges are grouped as
    [A-edges (src < split) | pad | B-edges | pad], each padded up to a
    multiple of 128 with (src=0, val=0, tgt_off=PAD_OFF).  Chunk counts are
    maxed over cores so one SPMD program fits all cores.
    """
    tgt = np.asarray(indices[0], dtype=np.int64).astype(np.int32)
    src = np.asarray(indices[1], dtype=np.int64).astype(np.int32)
    val = np.asarray(values, dtype=np.float32)

    core_of = np.minimum(tgt // own, n_cores - 1)

    per_core = []
    for k in range(n_cores):
        sel = np.nonzero(core_of == k)[0]
        tl = tgt[sel] - k * own
        order = np.argsort(tl, kind="stable")
        sel = sel[order]
        tl = tl[order]
        tile_id = tl >> 7
        bounds = np.searchsorted(tile_id, np.arange(ntiles + 1))
        tiles = []
        for t in range(ntiles):
            e = sel[bounds[t]:bounds[t + 1]]
            isa = src[e] < split
            tiles.append((e[isa], e[~isa]))
        per_core.append(tiles)

    ncha = [max(_ceil(max(len(per_core[k][t][0]) for k in range(n_cores)), 128), 1)
            for t in range(ntiles)]
    nchb = [_ceil(max(len(per_core[k][t][1]) for k in range(n_cores)), 128)
            for t in range(ntiles)]
    ch = [a + b for a, b in zip(ncha, nchb)]
    cht = sum(ch)
    fa = sum(ncha) * 8   # int16 idx cols (128 idx -> 8 cols of 16)
    fb = sum(nchb) * 8

    def wrap_idx(vals16, out, col0):
        # linear idx i -> (partition i%16 [+16*g replicas], col i//16)
        m = len(vals16) // 16
        blk = vals16.reshape(m, 16).T  # [16, m]
        for g in range(8):
            out[g * 16:(g + 1) * 16, col0:col0 + m] = blk

    cores = []
    for k in range(n_cores):
        idxa = np.zeros((128, fa), np.int16)
        idxb = np.zeros((128, max(fb, 1)), np.int16)
        toff = np.full((128, cht), PAD_OFF, np.float32)
        vals = np.zeros((128, cht), np.float32)
        toffi = np.zeros((128, cht * 8), np.int16)
        ca = cb = cc = 0
        for t in range(ntiles):
            ea, eb = per_core[k][t]
            for which, e, nch in (("a", ea, ncha[t]), ("b", eb, nchb[t])):
                nslot = nch * 128
                s = np.zeros(nslot, np.int32)
                s[:len(e)] = src[e] if which == "a" else src[e] - split
                to = np.full(nslot, PAD_OFF, np.float32)
                to[:len(e)] = (tgt[e] - k * own - t * 128).astype(np.float32)
                vv = np.zeros(nslot, np.float32)
                vv[:len(e)] = val[e]
                # chunk-major [p, c] layout: slot i -> (i % 128, i // 128)
                cols = slice(cc, cc + nch)
                toff[:, cols] = to.reshape(nch, 128).T
                vals[:, cols] = vv.reshape(nch, 128).T
                ti = np.zeros(nslot, np.int32)
                ti[:len(e)] = (tgt[e] - k * own - t * 128).astype(np.int32)
                wrap_idx(ti.astype(np.int16), toffi, cc * 8)
                cc += nch
                if which == "a":
                    wrap_idx(s.astype(np.int16), idxa, ca * 8)
                    ca += nch
                else:
                    if nch:
                        wrap_idx(s.astype(np.int16), idxb, cb * 8)
                    cb += nch
        cores.append(dict(idxa=idxa, idxb=idxb, toff=toff, val=vals,
                          toffi=toffi))

    meta = dict(ncha=ncha, nchb=nchb, ch=ch, cht=cht, fa=fa, fb=max(fb, 1),
                ownp=ntiles * 128)
    return meta, cores


# --------------------------------------------------------------------------
# device program
# --------------------------------------------------------------------------

def psum2_tile(p, tag):
    # one full PSUM bank (2 KB/partition) so each matmul accumulation group
    # owns its zero region exclusively
    return p.tile([128, 512], F32, tag=tag, name=f"pb_{tag}")


def _build_program(n, npad, d, n_cores, own, ntiles, split, meta_l, meta_u):
    nc = bacc.Bacc(trn_type="TRN2", target_bir_lowering=False, debug=False,
                   num_devices=n_cores, num_swdge_queues=4)
    ownp = ntiles * 128
    nblk = npad // 128          # 128-node score blocks
    J = 1792                    # phase-1 xt chunk cols (14 blocks)
    assert npad % J == 0
    niter = npad // J

    def din(name, shape, dt):
        return nc.dram_tensor(name, shape, dt, kind="ExternalInput")

    xs_tab = din("xs_tab", [npad, ROW], BF16)
    xt2 = din("xt2", [128, 2, npad], BF16)
    xot2 = din("xot2", [128, 2, ownp], BF16)
    wa42 = din("wa42", [128, 2, 4], BF16)
    w_l2 = din("w_l2", [128, 2, d], BF16)
    w_u2 = din("w_u2", [128, 2, d], BF16)
    w_lin2 = din("w_lin2", [128, 2, d], BF16)
    iota_in = din("iota_in", [128, 128], BF16)
    convs = {}
    for cv, meta in (("l", meta_l), ("u", meta_u)):
        convs[cv] = dict(
            meta=meta,
            idxa=din(f"idxa_{cv}", [128, meta["fa"]], I16),
            idxb=din(f"idxb_{cv}", [128, meta["fb"]], I16),
            toff=din(f"toff_{cv}", [128, meta["cht"]], F32),
            val=din(f"val_{cv}", [128, meta["cht"]], F32),
            toffi=din(f"toffi_{cv}", [128, meta["cht"] * 8], I16),
        )
    st_tab = nc.dram_tensor("st_tab", [ownp, 64], F32)
    out = nc.dram_tensor("out", [ownp, d], F32, kind="ExternalOutput")

    gmax = 8   # max chunks (x128 idx) per dma_gather call (SWDGE ring)

    def split_gather(out_tile, co, nch, table, idx_sb, io, elem, qiter):
        g0 = 0
        while g0 < nch:
            g = min(gmax, nch - g0)
            nc.gpsimd.dma_gather(
                out_tile[:, co + g0:co + g0 + g, :], table,
                idx_sb[:, (io + g0) * 8:(io + g0 + g) * 8],
                g * 128, g * 128, elem, elem_step=elem, queue_num=qiter[0] % 4)
            qiter[0] += 1
            g0 += g

    with tile.TileContext(nc) as tc:
        with ExitStack() as ctx:
            pool = ctx.enter_context(tc.tile_pool(name="sb", bufs=2))
            mpool = ctx.enter_context(tc.tile_pool(name="meta", bufs=1))
            cpool = ctx.enter_context(tc.tile_pool(name="const", bufs=1))

            iota_t = cpool.tile([128, 128], BF16)
            nc.sync.dma_start(iota_t[:], iota_in[:, :])
            ones_t = cpool.tile([128, 1], BF16)
            nc.vector.memset(ones_t[:], 1.0)
            wa_sb = cpool.tile([128, 2, 4], BF16)
            nc.sync.dma_start(wa_sb[:], wa42[:, :, :])
            xo_sb = cpool.tile([128, 2, ownp], BF16)
            nc.sync.dma_start(xo_sb[:], xot2[:, :, :])

            # ---- phase 1: scores, node-major on PE -----------------------
            p1cm = tc.tile_pool(name="p1", bufs=2)
            p1 = p1cm.__enter__()
            sc_sb = p1.tile([128, nblk, 4], F32, tag="scsb", bufs=1)
            with tc.tile_pool(name="ps1", bufs=2, space="PSUM") as ps1:
                for it in range(niter):
                    xt_t = p1.tile([128, 2, J], BF16, tag="xt")
                    nc.sync.dma_start(xt_t[:], xt2[:, :, it * J:(it + 1) * J])
                    scp = ps1.tile([128, J // 128, 4], F32, tag="scp", bufs=2)
                    for b in range(J // 128):
                        for h in range(2):
                            nc.tensor.matmul(
                                out=scp[:, b, :],
                                lhsT=xt_t[:, h, b * 128:(b + 1) * 128],
                                rhs=wa_sb[:, h, :],
                                start=(h == 0), stop=(h == 1))
                    nc.vector.tensor_copy(
                        out=sc_sb[:, it * (J // 128):(it + 1) * (J // 128), :],
                        in_=scp[:])
                # ss pair -> xs_tab cols [SS0, SS0+4) as raw f32-pair bytes
                nc.sync.dma_start(
                    xs_tab[0:npad, SS0:SS0 + 4].rearrange(
                        "(c p) f -> p c f", p=128),
                    sc_sb[:, :, 0:2].bitcast(BF16))

                # own-target st pair from resident xot2
                stp = ps1.tile([128, ntiles, 2], F32, tag="stp", bufs=1)
                for b in range(ntiles):
                    for h in range(2):
                        nc.tensor.matmul(
                            out=stp[:, b, :],
                            lhsT=xo_sb[:, h, b * 128:(b + 1) * 128],
                            rhs=wa_sb[:, h, 2:4],
                            start=(h == 0), stop=(h == 1))
                sto_sb = p1.tile([128, ntiles, 2], F32, tag="sto")
                nc.vector.tensor_copy(out=sto_sb[:], in_=stp[:])
                nc.sync.dma_start(
                    st_tab[0:ownp, 0:2].rearrange("(c p) f -> p c f", p=128),
                    sto_sb[:])

            wsb = {}
            for nm, w_in in (("l", w_l2), ("u", w_u2), ("x", w_lin2)):
                tl = cpool.tile([128, 2, d], BF16, tag=f"w{nm}",
                                name=f"w_{nm}")
                nc.sync.dma_start(tl[:], w_in[:, :, :])
                wsb[nm] = tl

            p1cm.__exit__(None, None, None)
            tc.strict_bb_all_engine_barrier()

            # per-conv metadata mega-loads
            msb = {}
            for cv in ("l", "u"):
                cd = convs[cv]
                meta = cd["meta"]
                d_ = {}
                d_["ia"] = mpool.tile([128, meta["fa"]], I16, tag=f"ia{cv}",
                                      name=f"ia_{cv}")
                nc.sync.dma_start(d_["ia"][:], cd["idxa"][:, :])
                d_["ib"] = mpool.tile([128, meta["fb"]], I16, tag=f"ib{cv}",
                                      name=f"ib_{cv}")
                nc.sync.dma_start(d_["ib"][:], cd["idxb"][:, :])
                d_["ti"] = mpool.tile([128, meta["cht"] * 8], I16,
                                      tag=f"ti{cv}", name=f"ti_{cv}")
                nc.sync.dma_start(d_["ti"][:], cd["toffi"][:, :])
                d_["tof"] = mpool.tile([128, meta["cht"]], F32, tag=f"tof{cv}",
                                       name=f"tof_{cv}")
                nc.sync.dma_start(d_["tof"][:], cd["toff"][:, :])
                d_["val"] = mpool.tile([128, meta["cht"]], F32, tag=f"val{cv}",
                                       name=f"val_{cv}")
                nc.sync.dma_start(d_["val"][:], cd["val"][:, :])
                msb[cv] = d_
            # ---- phase 2+3 interleaved: per tile, conv l then u,
            # then immediate projection + combine + relu ------------------
            qiter = [0]
            ps2 = ctx.enter_context(tc.tile_pool(name="ps2", bufs=1,
                                                 space="PSUM"))
            ps3 = ctx.enter_context(tc.tile_pool(name="ps3", bufs=2,
                                                 space="PSUM"))
            p3 = ctx.enter_context(tc.tile_pool(name="p3", bufs=2))
            offs = {}
            for cv in ("l", "u"):
                meta = convs[cv]["meta"]
                offs[cv] = (np.cumsum([0] + meta["ncha"]),
                            np.cumsum([0] + meta["nchb"]),
                            np.cumsum([0] + meta["ch"]))
            OB = 7  # tiles per output write batch (49 = 7*7)
            ost = None
            for t in range(ntiles):
                uts = {}
                rss = {}
                for cvi, cv in enumerate(("l", "u")):
                    meta = convs[cv]["meta"]
                    na, nb = meta["ncha"][t], meta["nchb"][t]
                    ch = meta["ch"][t]
                    offa, offb, offc = offs[cv]
                    c0 = offc[t]
                    ia_sb, ib_sb = msb[cv]["ia"], msb[cv]["ib"]
                    ti_sb = msb[cv]["ti"]
                    tof_sb, val_sb = msb[cv]["tof"], msb[cv]["val"]

                    xg = pool.tile([128, ch, ROW], BF16, tag=f"xg{cv}",
                                   name=f"xg_{cv}")
                    split_gather(xg, 0, na, xs_tab[:, :], ia_sb, offa[t],
                                 ROW, qiter)
                    if nb:
                        split_gather(xg, na, nb, xs_tab[split:, :], ib_sb,
                                     offb[t], ROW, qiter)
                    stg = pool.tile([128, ch, 64], F32, tag=f"stg{cv}",
                                    name=f"stg_{cv}")
                    split_gather(stg, 0, ch, st_tab[t * 128:, :], ti_sb,
                                 c0, 64, qiter)

                    ss = xg[:, :, SS0 + 2 * cvi:SS0 + 2 * cvi + 2].bitcast(F32)
                    st = stg[:, :, cvi:cvi + 1]
                    tof = tof_sb[:, c0:c0 + ch]
                    vv = val_sb[:, c0:c0 + ch]

                    # w = exp(elu(st+ss) * val); elu(z)=max(z,0)-1+e^min(z,0)
                    z = pool.tile([128, ch], F32, tag=f"z{cv}",
                                  name=f"z_{cv}")
                    nc.vector.tensor_add(out=z[:], in0=st, in1=ss)
                    zm = pool.tile([128, ch], F32, tag=f"zm{cv}",
                                   name=f"zm_{cv}")
                    nc.vector.tensor_scalar(out=zm[:], in0=z[:], scalar1=0.0,
                                            scalar2=None, op0=ALU.min)
                    e1 = pool.tile([128, ch], F32, tag=f"e1{cv}",
                                   name=f"e1_{cv}")
                    nc.scalar.activation(e1[:], zm[:], ACTF.Exp)
                    zp = pool.tile([128, ch], F32, tag=f"zp{cv}",
                                   name=f"zp_{cv}")
                    nc.vector.tensor_scalar(out=zp[:], in0=z[:], scalar1=0.0,
                                            scalar2=-1.0, op0=ALU.max,
                                            op1=ALU.add)
                    t1 = pool.tile([128, ch], F32, tag=f"t1{cv}",
                                   name=f"t1_{cv}")
                    nc.vector.tensor_add(out=t1[:], in0=zp[:], in1=e1[:])
                    v = pool.tile([128, ch], F32, tag=f"v{cv}",
                                  name=f"v_{cv}")
                    nc.vector.tensor_mul(out=v[:], in0=t1[:], in1=vv)
                    w = pool.tile([128, ch], F32, tag=f"w{cv}",
                                  name=f"w_{cv}")
                    nc.scalar.activation(w[:], v[:], ACTF.Exp)

                    ut0 = psum2_tile(ps2, "ut0")
                    ut1 = psum2_tile(ps2, "ut1")
                    s_ps = psum2_tile(ps2, "sps")
                    for c in range(ch):
                        mw = pool.tile([128, 128], BF16, tag=f"mw{cv}",
                                       name=f"mw_{cv}", bufs=8)
                        nc.vector.tensor_scalar(
                            out=mw[:], in0=iota_t[:], scalar1=tof[:, c:c + 1],
                            scalar2=w[:, c:c + 1], op0=ALU.is_equal,
                            op1=ALU.mult)
                        nc.tensor.matmul(
                            out=ut0[:, 0:128], lhsT=xg[:, c, 0:128], rhs=mw[:],
                            start=(c == 0), stop=(c == ch - 1))
                        nc.tensor.matmul(
                            out=ut1[:, 0:128], lhsT=xg[:, c, 128:256],
                            rhs=mw[:],
                            start=(c == 0), stop=(c == ch - 1))
                        nc.tensor.matmul(out=s_ps[:, 0:1], lhsT=mw[:],
                                         rhs=ones_t[:],
                                         start=(c == 0), stop=(c == ch - 1))
                    atk = pool.tile([128, 2, 128], BF16, tag=f"at{cv}",
                                    name=f"at_{cv}")
                    nc.vector.tensor_copy(out=atk[:, 0, :], in_=ut0[:, 0:128])
                    nc.vector.tensor_copy(out=atk[:, 1, :], in_=ut1[:, 0:128])
                    uts[cv] = atk
                    sden = pool.tile([128, 1], F32, tag=f"sden{cv}",
                                     name=f"sden_{cv}")
                    nc.vector.tensor_scalar(out=sden[:], in0=s_ps[:, 0:1],
                                            scalar1=1e-30, scalar2=None,
                                            op0=ALU.max)
                    rsk = pool.tile([128, 1], F32, tag=f"rs{cv}",
                                    name=f"rs_{cv}")
                    nc.vector.reciprocal(rsk[:], sden[:])
                    rss[cv] = rsk

                # ---- phase 3 for tile t --------------------------------
                i = t % OB
                if i == 0:
                    ost = p3.tile([128, OB, d], F32, tag="ost")
                tc0, tc1 = t * 128, (t + 1) * 128
                o123 = ps3.tile([128, 3, d], F32, tag="o123")
                for j, (atk, wk) in enumerate(
                        ((uts["l"], "l"), (uts["u"], "u"), (None, "x"))):
                    for h in range(2):
                        lhs = (atk[:, h, :] if atk is not None
                               else xo_sb[:, h, tc0:tc1])
                        nc.tensor.matmul(out=o123[:, j, :], lhsT=lhs,
                                         rhs=wsb[wk][:, h, :],
                                         start=(h == 0), stop=(h == 1))
                a1 = p3.tile([128, d], F32, tag="a1")
                nc.vector.tensor_scalar(out=a1[:], in0=o123[:, 0, :],
                                        scalar1=rss["l"][:], scalar2=None,
                                        op0=ALU.mult)
                a2 = p3.tile([128, d], F32, tag="a2")
                nc.vector.tensor_scalar(out=a2[:], in0=o123[:, 1, :],
                                        scalar1=rss["u"][:], scalar2=None,
                                        op0=ALU.mult)
                a3 = p3.tile([128, d], F32, tag="a1")
                nc.vector.tensor_add(out=a3[:], in0=a1[:], in1=a2[:])
                a4 = p3.tile([128, d], F32, tag="a2")
                nc.vector.tensor_add(out=a4[:], in0=a3[:], in1=o123[:, 2, :])
                nc.scalar.activation(ost[:, i, :], a4[:], ACTF.Relu)
                if i == OB - 1 or t == ntiles - 1:
                    t0 = t - i
                    nc.sync.dma_start(
                        out[t0 * 128:(t + 1) * 128, :].rearrange(
                            "(b p) f -> p b f", p=128),
                        ost[:, 0:i + 1, :])
    import re as _re
    for blk in nc.m.functions[0].blocks:
        for inst in blk.instructions:
            if isinstance(inst, mybir.InstDMAGatherAnt):
                lane = None
                si = inst.sync_info
                ups = si.on_update if si is not None else []
                for u in ups:
                    m = _re.search(r"DMASW(\d+)", str(u.ant_name))
                    if m:
                        lane = int(m.group(1))
                        break
                if lane is not None:
                    inst.queue_num = lane % 4
    nc.finalize()
    return nc


# --------------------------------------------------------------------------
# top level
# --------------------------------------------------------------------------

def _prepare(x, lower_indices, lower_values, upper_indices, upper_values,
             W_lower, att_lower, W_upper, att_upper, W_lin,
             n_cores=8, split=32768):
    n, d = x.shape
    ntiles = _ceil(_ceil(n, n_cores), 128)
    own = ntiles * 128          # 128-aligned ownership (6272)
    ownp = own
    npad = _ceil(max(n_cores * ownp, n), 512) * 512

    meta_l, cores_l = _prep_conv(lower_indices, lower_values, n, n_cores, own,
                                 ntiles, split)
    meta_u, cores_u = _prep_conv(upper_indices, upper_values, n, n_cores, own,
                                 ntiles, split)

    xf = np.asarray(x, np.float32)
    x_pad = np.zeros((npad, d), np.float32)
    x_pad[:n] = xf
    xs_np = np.zeros((npad, ROW), ml_dtypes.bfloat16)
    xs_np[:, :d] = x_pad.astype(ml_dtypes.bfloat16)
    xt_b = np.ascontiguousarray(x_pad.T).astype(ml_dtypes.bfloat16)  # [d, npad]
    xt2 = np.ascontiguousarray(
        xt_b.reshape(2, 128, npad).transpose(1, 0, 2))  # [128, 2, npad]

    wl = np.asarray(W_lower, np.float32)
    wu = np.asarray(W_upper, np.float32)
    wlin = np.asarray(W_lin, np.float32) * EPS
    al = np.asarray(att_lower, np.float32)
    au = np.asarray(att_upper, np.float32)
    # wa4 cols -> fields: [ss_l, ss_u, st_l, st_u]
    wa4 = np.stack([wl @ al[d:], wu @ au[d:], wl @ al[:d], wu @ au[:d]],
                   axis=1).astype(ml_dtypes.bfloat16)   # [d, 4]
    wa42 = np.ascontiguousarray(wa4.reshape(2, 128, 4).transpose(1, 0, 2))

    def w2(w):
        return np.ascontiguousarray(
            w.astype(ml_dtypes.bfloat16).reshape(2, 128, d).transpose(1, 0, 2))

    iota_np = np.broadcast_to(np.arange(128), (128, 128)).astype(
        ml_dtypes.bfloat16)

    common = dict(xs_tab=xs_np, xt2=xt2, wa42=wa42,
                  w_l2=w2(wl), w_u2=w2(wu), w_lin2=w2(wlin),
                  iota_in=iota_np)
    in_maps = []
    for k in range(n_cores):
        m = dict(common)
        xo = np.ascontiguousarray(x_pad[k * own:k * own + ownp].T).astype(
            ml_dtypes.bfloat16)
        m["xot2"] = np.ascontiguousarray(
            xo.reshape(2, 128, ownp).transpose(1, 0, 2))
        for cv, cores in (("l", cores_l), ("u", cores_u)):
            cdk = cores[k]
            m[f"idxa_{cv}"] = cdk["idxa"]
            m[f"idxb_{cv}"] = cdk["idxb"]
            m[f"toff_{cv}"] = cdk["toff"]
            m[f"val_{cv}"] = cdk["val"]
            m[f"toffi_{cv}"] = cdk["toffi"]
        in_maps.append(m)

    dims = dict(n=n, npad=npad, d=d, n_cores=n_cores, own=own, ntiles=ntiles,
                split=split)
    return dims, meta_l, meta_u, in_maps


def build_all(inputs, n_cores=8, split=32768):
    dims, meta_l, meta_u, in_maps = _prepare(**inputs, n_cores=n_cores,
                                             split=split)
    nc = _build_program(dims["n"], dims["npad"], dims["d"], dims["n_cores"],
                        dims["own"], dims["ntiles"], dims["split"],
                        meta_l, meta_u)
    return nc, in_maps, dims


def kernel(**inputs):
    nc, in_maps, dims = build_all(inputs)
    res = run_bass_kernel_spmd(nc, in_maps, list(range(dims["n_cores"])))
    outs = [res.results[k]["out"][:dims["own"]] for k in range(dims["n_cores"])]
    return np.concatenate(outs, axis=0)[:dims["n"]].astype(np.float32)


# revision 4
# speedup vs baseline: 1.2319x; 1.1594x over previous
"""Trainium2 Bass kernel for nn_CANLayer (CAN GNN layer) — v2.

Strategy (8-core SPMD, no collectives):
  - Targets sharded 6272/core (49 tiles of 128, 128-aligned ownership).
    Edges routed to the target-owning core; every softmax segment is local.
  - Phase 1 (scores): node-major matvec on PE (lhsT = x^T blocks, rhs = the
    four projected attention vectors); the ss pair is written into the
    combined gather table xs_tab cols 256:260 (f32 pair packed in bf16
    cols) in one descriptor-heavy DMA, the st pair for own targets into
    st_tab.  No transposes, no zeroing passes.
  - Combined gather row (768B): [x bf16(256) | ss_l,ss_u f32(8B) | pad].
    Gather calls are merged across the two convs per tile (A_l|A_u and
    B_l|B_u index streams share one output buffer; st indices for both
    convs share one call stream), <=1024 indices per call (HW SWDGE ring
    cap).  st gathered from st_tab (256B rows) with tile-local indices.
  - Per 128-edge chunk: fused one-hot build maskw[e,t] = (iota==tof)*w with
    w = exp(elu(st+ss)*val) computed in 6 batched ops; then
    U^T[d,t] += matmul(lhsT=xg d-half, rhs=maskw) into per-bank PSUM and
    s[t] += matmul(lhsT=maskw, rhs=ones) (s as a column for per-partition
    scaling later).  Unnormalized U^T copied to bf16 per tile.
  - Convs l/u interleaved per tile; phase 3 fused into the tile loop:
    out = relu(rs_l*(U_l^T.T@W_l) + rs_u*(U_u^T.T@W_u) + x@(W_lin*EPS))
    with rs = 1/max(s,eps) per-partition scalars, batched 7-tile output
    writes.  Whole-conv idx/toff/val metadata loaded in a few mega-DMAs.
"""

import os
import sys
from contextlib import ExitStack

import numpy as np
import ml_dtypes

for _p in ("/opt/trn_rl_repo", "/root/.axon_site/_ro/trn_rl_repo"):
    if os.path.isdir(_p) and _p not in sys.path:
        sys.path.insert(0, _p)

import concourse.bass as bass
import concourse.bacc as bacc
import concourse.tile as tile
from concourse import mybir
from concourse.bass_utils import run_bass_kernel_spmd

BF16 = mybir.dt.bfloat16
FP8 = mybir.dt.float8e4
F32 = mybir.dt.float32
I16 = mybir.dt.int16
ALU = mybir.AluOpType
ACTF = mybir.ActivationFunctionType

EPS = 1.0 + 1e-06
PAD_OFF = 200.0  # tgt_off value for pad edge slots (outside [0,128) window)
ROW = 384        # xs_tab row cols (bf16) = 768 bytes
SS0 = 256        # first score col in xs_tab


def _ceil(a, b):
    return -(-a // b)


# --------------------------------------------------------------------------
# host-side preprocessing
# --------------------------------------------------------------------------

def _prep_conv(indices, values, n, n_cores, own, ntiles, split):
    """Sort/tile/pad one conv's edges.  Returns per-core arrays + baked meta.

    Per tile (128 consecutive owned targets): edges are grouped as
    [A-edges (src < split) | pad | B-edges | pad], each padded up to a
    multiple of 128 with (src=0, val=0, tgt_off=PAD_OFF).  Chunk counts are
    maxed over cores so one SPMD program fits all cores.
    """
    tgt = np.asarray(indices[0], dtype=np.int64).astype(np.int32)
    src = np.asarray(indices[1], dtype=np.int64).astype(np.int32)
    val = np.asarray(values, dtype=np.float32)

    core_of = np.minimum(tgt // own, n_cores - 1)

    per_core = []
    for k in range(n_cores):
        sel = np.nonzero(core_of == k)[0]
        tl = tgt[sel] - k * own
        order = np.argsort(tl, kind="stable")
        sel = sel[order]
        tl = tl[order]
        tile_id = tl >> 7
        bounds = np.searchsorted(tile_id, np.arange(ntiles + 1))
        tiles = []
        for t in range(ntiles):
            e = sel[bounds[t]:bounds[t + 1]]
            isa = src[e] < split
            tiles.append((e[isa], e[~isa]))
        per_core.append(tiles)

    ncha = [max(_ceil(max(len(per_core[k][t][0]) for k in range(n_cores)), 128), 1)
            for t in range(ntiles)]
    nchb = [_ceil(max(len(per_core[k][t][1]) for k in range(n_cores)), 128)
            for t in range(ntiles)]
    ch = [a + b for a, b in zip(ncha, nchb)]
    cht = sum(ch)
    fa = sum(ncha) * 8   # int16 idx cols (128 idx -> 8 cols of 16)
    fb = sum(nchb) * 8

    def wrap_idx(vals16, out, col0):
        # linear idx i -> (partition i%16 [+16*g replicas], col i//16)
        m = len(vals16) // 16
        blk = vals16.reshape(m, 16).T  # [16, m]
        for g in range(8):
            out[g * 16:(g + 1) * 16, col0:col0 + m] = blk

    cores = []
    for k in range(n_cores):
        idxa = np.zeros((128, fa), np.int16)
        idxb = np.zeros((128, max(fb, 1)), np.int16)
        toff = np.full((128, cht), PAD_OFF, np.float32)
        vals = np.zeros((128, cht), np.float32)
        toffi = np.zeros((128, cht * 8), np.int16)
        ca = cb = cc = 0
        for t in range(ntiles):
            ea, eb = per_core[k][t]
            for which, e, nch in (("a", ea, ncha[t]), ("b", eb, nchb[t])):
                nslot = nch * 128
                s = np.zeros(nslot, np.int32)
                s[:len(e)] = src[e] if which == "a" else src[e] - split
                to = np.full(nslot, PAD_OFF, np.float32)
                to[:len(e)] = (tgt[e] - k * own - t * 128).astype(np.float32)
                vv = np.zeros(nslot, np.float32)
                vv[:len(e)] = val[e]
                # chunk-major [p, c] layout: slot i -> (i % 128, i // 128)
                cols = slice(cc, cc + nch)
                toff[:, cols] = to.reshape(nch, 128).T
                vals[:, cols] = vv.reshape(nch, 128).T
                ti = np.zeros(nslot, np.int32)
                ti[:len(e)] = (tgt[e] - k * own - t * 128).astype(np.int32)
                wrap_idx(ti.astype(np.int16), toffi, cc * 8)
                cc += nch
                if which == "a":
                    wrap_idx(s.astype(np.int16), idxa, ca * 8)
                    ca += nch
                else:
                    if nch:
                        wrap_idx(s.astype(np.int16), idxb, cb * 8)
                    cb += nch
        cores.append(dict(idxa=idxa, idxb=idxb, toff=toff, val=vals,
                          toffi=toffi))

    meta = dict(ncha=ncha, nchb=nchb, ch=ch, cht=cht, fa=fa, fb=max(fb, 1),
                ownp=ntiles * 128)
    return meta, cores


# --------------------------------------------------------------------------
# device program
# --------------------------------------------------------------------------

def psum2_tile(p, tag):
    # one full PSUM bank (2 KB/partition) so each matmul accumulation group
    # owns its zero region exclusively
    return p.tile([128, 512], F32, tag=tag, name=f"pb_{tag}")


def _build_program(n, npad, d, n_cores, own, ntiles, split, meta_l, meta_u):
    nc = bacc.Bacc(trn_type="TRN2", target_bir_lowering=False, debug=False,
                   num_devices=n_cores, num_swdge_queues=4)
    ownp = ntiles * 128
    nblk = npad // 128          # 128-node score blocks
    J = 1792                    # phase-1 xt chunk cols (14 blocks)
    assert npad % J == 0
    niter = npad // J

    def din(name, shape, dt):
        return nc.dram_tensor(name, shape, dt, kind="ExternalInput")

    xs_tab = din("xs_tab", [npad, ROW], BF16)
    xt2 = din("xt2", [128, 2, npad], BF16)
    xot2 = din("xot2", [128, 2, ownp], BF16)
    wa42 = din("wa42", [128, 2, 4], BF16)
    w_l2 = din("w_l2", [128, 2, d], BF16)
    w_u2 = din("w_u2", [128, 2, d], BF16)
    w_lin2 = din("w_lin2", [128, 2, d], BF16)
    iota_in = din("iota_in", [128, 128], BF16)
    convs = {}
    for cv, meta in (("l", meta_l), ("u", meta_u)):
        convs[cv] = dict(
            meta=meta,
            toff=din(f"toff_{cv}", [128, meta["cht"]], F32),
            val=din(f"val_{cv}", [128, meta["cht"]], F32),
        )
    toffi_c = din("toffi_c", [128, meta_l["cht_c"] * 8], I16)
    idxa_c = din("idxa_c", [128, meta_l["fa_c"]], I16)
    idxb_c = din("idxb_c", [128, meta_l["fb_c"]], I16)
    st_tab = nc.dram_tensor("st_tab", [ownp, 64], F32)
    out = nc.dram_tensor("out", [ownp, d], F32, kind="ExternalOutput")

    gmax = 8   # max chunks (x128 idx) per dma_gather call (SWDGE ring)

    def split_gather(out_tile, co, nch, table, idx_sb, io, elem, qiter):
        g0 = 0
        while g0 < nch:
            g = min(gmax, nch - g0)
            nc.gpsimd.dma_gather(
                out_tile[:, co + g0:co + g0 + g, :], table,
                idx_sb[:, (io + g0) * 8:(io + g0 + g) * 8],
                g * 128, g * 128, elem, elem_step=elem, queue_num=qiter[0] % 4)
            qiter[0] += 1
            g0 += g

    with tile.TileContext(nc) as tc:
        with ExitStack() as ctx:
            pool = ctx.enter_context(tc.tile_pool(name="sb", bufs=2))
            mpool = ctx.enter_context(tc.tile_pool(name="meta", bufs=1))
            cpool = ctx.enter_context(tc.tile_pool(name="const", bufs=1))

            iota_t = cpool.tile([128, 128], BF16)
            nc.sync.dma_start(iota_t[:], iota_in[:, :])
            ones_t = cpool.tile([128, 1], BF16)
            nc.vector.memset(ones_t[:], 1.0)
            wa_sb = cpool.tile([128, 2, 4], BF16)
            nc.sync.dma_start(wa_sb[:], wa42[:, :, :])
            xo_sb = cpool.tile([128, 2, ownp], BF16)
            nc.sync.dma_start(xo_sb[:], xot2[:, :, :])

            # ---- phase 1: scores, node-major on PE -----------------------
            p1cm = tc.tile_pool(name="p1", bufs=2)
            p1 = p1cm.__enter__()
            sc_sb = p1.tile([128, nblk, 4], F32, tag="scsb", bufs=1)
            with tc.tile_pool(name="ps1", bufs=2, space="PSUM") as ps1:
                for it in range(niter):
                    xt_t = p1.tile([128, 2, J], BF16, tag="xt")
                    nc.sync.dma_start(xt_t[:], xt2[:, :, it * J:(it + 1) * J])
                    scp = ps1.tile([128, J // 128, 4], F32, tag="scp", bufs=2)
                    for b in range(J // 128):
                        for h in range(2):
                            nc.tensor.matmul(
                                out=scp[:, b, :],
                                lhsT=xt_t[:, h, b * 128:(b + 1) * 128],
                                rhs=wa_sb[:, h, :],
                                start=(h == 0), stop=(h == 1))
                    nc.vector.tensor_copy(
                        out=sc_sb[:, it * (J // 128):(it + 1) * (J // 128), :],
                        in_=scp[:])
                # ss pair -> xs_tab cols [SS0, SS0+4) as raw f32-pair bytes
                nc.sync.dma_start(
                    xs_tab[0:npad, SS0:SS0 + 4].rearrange(
                        "(c p) f -> p c f", p=128),
                    sc_sb[:, :, 0:2].bitcast(BF16))

                # own-target st pair from resident xot2
                stp = ps1.tile([128, ntiles, 2], F32, tag="stp", bufs=1)
                for b in range(ntiles):
                    for h in range(2):
                        nc.tensor.matmul(
                            out=stp[:, b, :],
                            lhsT=xo_sb[:, h, b * 128:(b + 1) * 128],
                            rhs=wa_sb[:, h, 2:4],
                            start=(h == 0), stop=(h == 1))
                sto_sb = p1.tile([128, ntiles, 2], F32, tag="sto")
                nc.vector.tensor_copy(out=sto_sb[:], in_=stp[:])
                nc.sync.dma_start(
                    st_tab[0:ownp, 0:2].rearrange("(c p) f -> p c f", p=128),
                    sto_sb[:])

            wsb = {}
            for nm, w_in in (("l", w_l2), ("u", w_u2), ("x", w_lin2)):
                tl = cpool.tile([128, 2, d], BF16, tag=f"w{nm}",
                                name=f"w_{nm}")
                nc.sync.dma_start(tl[:], w_in[:, :, :])
                wsb[nm] = tl

            p1cm.__exit__(None, None, None)
            tc.strict_bb_all_engine_barrier()

            # per-conv metadata mega-loads
            msb = {}
            for cv in ("l", "u"):
                cd = convs[cv]
                meta = cd["meta"]
                d_ = {}
                d_["tof"] = mpool.tile([128, meta["cht"]], F32, tag=f"tof{cv}",
                                       name=f"tof_{cv}")
                nc.sync.dma_start(d_["tof"][:], cd["toff"][:, :])
                d_["val"] = mpool.tile([128, meta["cht"]], F32, tag=f"val{cv}",
                                       name=f"val_{cv}")
                nc.sync.dma_start(d_["val"][:], cd["val"][:, :])
                msb[cv] = d_
            iac_sb = mpool.tile([128, meta_l["fa_c"]], I16, tag="iac")
            nc.sync.dma_start(iac_sb[:], idxa_c[:, :])
            ibc_sb = mpool.tile([128, meta_l["fb_c"]], I16, tag="ibc")
            nc.sync.dma_start(ibc_sb[:], idxb_c[:, :])
            # ---- phase 2+3 interleaved: per tile, conv l then u,
            # then immediate projection + combine + relu ------------------
            qiter = [0]
            ps2 = ctx.enter_context(tc.tile_pool(name="ps2", bufs=1,
                                                 space="PSUM"))
            ps3 = ctx.enter_context(tc.tile_pool(name="ps3", bufs=2,
                                                 space="PSUM"))
            p3 = ctx.enter_context(tc.tile_pool(name="p3", bufs=2))
            offs = {}
            for cv in ("l", "u"):
                meta = convs[cv]["meta"]
                offs[cv] = (np.cumsum([0] + meta["ncha"]),
                            np.cumsum([0] + meta["nchb"]),
                            np.cumsum([0] + meta["ch"]))
            OB = 7  # tiles per output write batch (49 = 7*7)
            ost = None
            offc_c = np.cumsum(
                [0] + [meta_l["ch"][t] + meta_u["ch"][t]
                       for t in range(ntiles)])
            offa_c = np.cumsum(
                [0] + [meta_l["ncha"][t] + meta_u["ncha"][t]
                       for t in range(ntiles)])
            offb_c = np.cumsum(
                [0] + [meta_l["nchb"][t] + meta_u["nchb"][t]
                       for t in range(ntiles)])
            for t in range(ntiles):
                uts = {}
                rss = {}
                ch_l = convs["l"]["meta"]["ch"][t]
                ch_c = ch_l + convs["u"]["meta"]["ch"][t]
                c0_c = offc_c[t]
                ti_sb = pool.tile([128, ch_c * 8], I16, tag="tic")
                nc.sync.dma_start(
                    ti_sb[:], toffi_c[:, c0_c * 8:(c0_c + ch_c) * 8])
                stga = pool.tile([128, ch_c, 64], F32, tag="stga")
                split_gather(stga, 0, ch_c, st_tab[t * 128:, :], ti_sb,
                             0, 64, qiter)
                na_l = convs["l"]["meta"]["ncha"][t]
                na_u = convs["u"]["meta"]["ncha"][t]
                nb_l = convs["l"]["meta"]["nchb"][t]
                nb_u = convs["u"]["meta"]["nchb"][t]
                naa = na_l + na_u
                nbb = nb_l + nb_u
                xga = pool.tile([128, naa + nbb, ROW], BF16, tag="xga")
                split_gather(xga, 0, naa, xs_tab[:, :], iac_sb, offa_c[t],
                             ROW, qiter)
                if nbb:
                    split_gather(xga, naa, nbb, xs_tab[split:, :], ibc_sb,
                                 offb_c[t], ROW, qiter)
                for cvi, cv in enumerate(("l", "u")):
                    meta = convs[cv]["meta"]
                    na, nb = meta["ncha"][t], meta["nchb"][t]
                    ch = meta["ch"][t]
                    offa, offb, offc = offs[cv]
                    c0 = offc[t]
                    tof_sb, val_sb = msb[cv]["tof"], msb[cv]["val"]

                    # chunk c of this conv -> column in the combined xga:
                    # layout [A_l | A_u | B_l | B_u]
                    a0 = 0 if cvi == 0 else na_l
                    b0 = naa + (0 if cvi == 0 else nb_l)
                    cols = [a0 + c if c < na else b0 + (c - na)
                            for c in range(ch)]
                    stg0 = 0 if cvi == 0 else ch_l
                    sslo = SS0 + 2 * cvi
                    ss_a = xga[:, a0:a0 + na, sslo:sslo + 2].bitcast(F32)
                    st = stga[:, stg0:stg0 + ch, cvi:cvi + 1]
                    tof = tof_sb[:, c0:c0 + ch]
                    vv = val_sb[:, c0:c0 + ch]

                    # w = exp(elu(st+ss) * val); elu(z)=max(z,0)-1+e^min(z,0)
                    z = pool.tile([128, ch], F32, tag=f"z{cv}",
                                  name=f"z_{cv}")
                    nc.vector.tensor_add(out=z[:, 0:na], in0=st[:, 0:na, :],
                                         in1=ss_a)
                    if nb:
                        ss_b = xga[:, b0:b0 + nb,
                                   sslo:sslo + 2].bitcast(F32)
                        nc.vector.tensor_add(out=z[:, na:ch],
                                             in0=st[:, na:ch, :], in1=ss_b)
                    zm = pool.tile([128, ch], F32, tag=f"zm{cv}",
                                   name=f"zm_{cv}")
                    nc.vector.tensor_scalar(out=zm[:], in0=z[:], scalar1=0.0,
                                            scalar2=None, op0=ALU.min)
                    e1 = pool.tile([128, ch], F32, tag=f"e1{cv}",
                                   name=f"e1_{cv}")
                    nc.scalar.activation(e1[:], zm[:], ACTF.Exp)
                    zp = pool.tile([128, ch], F32, tag=f"zp{cv}",
                                   name=f"zp_{cv}")
                    nc.vector.tensor_scalar(out=zp[:], in0=z[:], scalar1=0.0,
                                            scalar2=-1.0, op0=ALU.max,
                                            op1=ALU.add)
                    t1 = pool.tile([128, ch], F32, tag=f"t1{cv}",
                                   name=f"t1_{cv}")
                    nc.vector.tensor_add(out=t1[:], in0=zp[:], in1=e1[:])
                    v = pool.tile([128, ch], F32, tag=f"v{cv}",
                                  name=f"v_{cv}")
                    nc.vector.tensor_mul(out=v[:], in0=t1[:], in1=vv)
                    w = pool.tile([128, ch], F32, tag=f"w{cv}",
                                  name=f"w_{cv}")
                    nc.scalar.activation(w[:], v[:], ACTF.Exp)

                    ut0 = psum2_tile(ps2, "ut0")
                    ut1 = psum2_tile(ps2, "ut1")
                    s_ps = psum2_tile(ps2, "sps")
                    for c in range(ch):
                        mw = pool.tile([128, 128], BF16, tag=f"mw{cv}",
                                       name=f"mw_{cv}", bufs=8)
                        nc.vector.tensor_scalar(
                            out=mw[:], in0=iota_t[:], scalar1=tof[:, c:c + 1],
                            scalar2=w[:, c:c + 1], op0=ALU.is_equal,
                            op1=ALU.mult)
                        xc = cols[c]
                        nc.tensor.matmul(
                            out=ut0[:, 0:128], lhsT=xga[:, xc, 0:128],
                            rhs=mw[:],
                            start=(c == 0), stop=(c == ch - 1))
                        nc.tensor.matmul(
                            out=ut1[:, 0:128], lhsT=xga[:, xc, 128:256],
                            rhs=mw[:],
                            start=(c == 0), stop=(c == ch - 1))
                        nc.tensor.matmul(out=s_ps[:, 0:1], lhsT=mw[:],
                                         rhs=ones_t[:],
                                         start=(c == 0), stop=(c == ch - 1))
                    atk = pool.tile([128, 2, 128], BF16, tag=f"at{cv}",
                                    name=f"at_{cv}")
                    nc.vector.tensor_copy(out=atk[:, 0, :], in_=ut0[:, 0:128])
                    nc.vector.tensor_copy(out=atk[:, 1, :], in_=ut1[:, 0:128])
                    uts[cv] = atk
                    sden = pool.tile([128, 1], F32, tag=f"sden{cv}",
                                     name=f"sden_{cv}")
                    nc.vector.tensor_scalar(out=sden[:], in0=s_ps[:, 0:1],
                                            scalar1=1e-30, scalar2=None,
                                            op0=ALU.max)
                    rsk = pool.tile([128, 1], F32, tag=f"rs{cv}",
                                    name=f"rs_{cv}")
                    nc.vector.reciprocal(rsk[:], sden[:])
                    rss[cv] = rsk

                # ---- phase 3 for tile t --------------------------------
                i = t % OB
                if i == 0:
                    ost = p3.tile([128, OB, d], F32, tag="ost")
                tc0, tc1 = t * 128, (t + 1) * 128
                o123 = ps3.tile([128, 3, d], F32, tag="o123")
                for j, (atk, wk) in enumerate(
                        ((uts["l"], "l"), (uts["u"], "u"), (None, "x"))):
                    for h in range(2):
                        lhs = (atk[:, h, :] if atk is not None
                               else xo_sb[:, h, tc0:tc1])
                        nc.tensor.matmul(out=o123[:, j, :], lhsT=lhs,
                                         rhs=wsb[wk][:, h, :],
                                         start=(h == 0), stop=(h == 1))
                a1 = p3.tile([128, d], F32, tag="a1")
                nc.vector.tensor_scalar(out=a1[:], in0=o123[:, 0, :],
                                        scalar1=rss["l"][:], scalar2=None,
                                        op0=ALU.mult)
                a2 = p3.tile([128, d], F32, tag="a2")
                nc.vector.tensor_scalar(out=a2[:], in0=o123[:, 1, :],
                                        scalar1=rss["u"][:], scalar2=None,
                                        op0=ALU.mult)
                a3 = p3.tile([128, d], F32, tag="a1")
                nc.vector.tensor_add(out=a3[:], in0=a1[:], in1=a2[:])
                a4 = p3.tile([128, d], F32, tag="a2")
                nc.vector.tensor_add(out=a4[:], in0=a3[:], in1=o123[:, 2, :])
                nc.scalar.activation(ost[:, i, :], a4[:], ACTF.Relu)
                if i == OB - 1 or t == ntiles - 1:
                    t0 = t - i
                    nc.sync.dma_start(
                        out[t0 * 128:(t + 1) * 128, :].rearrange(
                            "(b p) f -> p b f", p=128),
                        ost[:, 0:i + 1, :])
    import re as _re
    for blk in nc.m.functions[0].blocks:
        for inst in blk.instructions:
            if isinstance(inst, mybir.InstDMAGatherAnt):
                lane = None
                si = inst.sync_info
                ups = si.on_update if si is not None else []
                for u in ups:
                    m = _re.search(r"DMASW(\d+)", str(u.ant_name))
                    if m:
                        lane = int(m.group(1))
                        break
                if lane is not None:
                    inst.queue_num = lane % 4
    nc.finalize()
    return nc


# --------------------------------------------------------------------------
# top level
# --------------------------------------------------------------------------

def _prepare(x, lower_indices, lower_values, upper_indices, upper_values,
             W_lower, att_lower, W_upper, att_upper, W_lin,
             n_cores=8, split=32768):
    n, d = x.shape
    ntiles = _ceil(_ceil(n, n_cores), 128)
    own = ntiles * 128          # 128-aligned ownership (6272)
    ownp = own
    npad = _ceil(max(n_cores * ownp, n), 512) * 512

    meta_l, cores_l = _prep_conv(lower_indices, lower_values, n, n_cores, own,
                                 ntiles, split)
    meta_u, cores_u = _prep_conv(upper_indices, upper_values, n, n_cores, own,
                                 ntiles, split)

    # combined st-gather indices: per tile, conv-l chunks then conv-u chunks
    offc_l = np.cumsum([0] + meta_l["ch"])
    offc_u = np.cumsum([0] + meta_u["ch"])
    cht_c = meta_l["cht"] + meta_u["cht"]
    # combined xs-gather indices: per tile, [A_l | A_u] and [B_l | B_u]
    offa_l = np.cumsum([0] + meta_l["ncha"])
    offa_u = np.cumsum([0] + meta_u["ncha"])
    offb_l = np.cumsum([0] + meta_l["nchb"])
    offb_u = np.cumsum([0] + meta_u["nchb"])
    fa_c = (sum(meta_l["ncha"]) + sum(meta_u["ncha"])) * 8
    fb_c = max((sum(meta_l["nchb"]) + sum(meta_u["nchb"])) * 8, 8)
    for k in range(n_cores):
        tic = np.zeros((128, cht_c * 8), np.int16)
        iac = np.zeros((128, fa_c), np.int16)
        ibc = np.zeros((128, fb_c), np.int16)
        col = ca = cb = 0
        for t in range(ntiles):
            for cores, offc, offa, offb, meta in (
                    (cores_l, offc_l, offa_l, offb_l, meta_l),
                    (cores_u, offc_u, offa_u, offb_u, meta_u)):
                chv = meta["ch"][t]
                tic[:, col * 8:(col + chv) * 8] = \
                    cores[k]["toffi"][:, offc[t] * 8:(offc[t] + chv) * 8]
                col += chv
            for cores, offa, meta in ((cores_l, offa_l, meta_l),
                                      (cores_u, offa_u, meta_u)):
                nav = meta["ncha"][t]
                iac[:, ca * 8:(ca + nav) * 8] = \
                    cores[k]["idxa"][:, offa[t] * 8:(offa[t] + nav) * 8]
                ca += nav
            for cores, offb, meta in ((cores_l, offb_l, meta_l),
                                      (cores_u, offb_u, meta_u)):
                nbv = meta["nchb"][t]
                if nbv:
                    ibc[:, cb * 8:(cb + nbv) * 8] = \
                        cores[k]["idxb"][:, offb[t] * 8:(offb[t] + nbv) * 8]
                cb += nbv
        cores_l[k]["toffi_c"] = tic
        cores_l[k]["idxa_c"] = iac
        cores_l[k]["idxb_c"] = ibc
    meta_l["cht_c"] = cht_c
    meta_l["fa_c"] = fa_c
    meta_l["fb_c"] = fb_c

    xf = np.asarray(x, np.float32)
    x_pad = np.zeros((npad, d), np.float32)
    x_pad[:n] = xf
    xs_np = np.zeros((npad, ROW), ml_dtypes.bfloat16)
    xs_np[:, :d] = x_pad.astype(ml_dtypes.bfloat16)
    xt_b = np.ascontiguousarray(x_pad.T).astype(ml_dtypes.bfloat16)  # [d, npad]
    xt2 = np.ascontiguousarray(
        xt_b.reshape(2, 128, npad).transpose(1, 0, 2))  # [128, 2, npad]

    wl = np.asarray(W_lower, np.float32)
    wu = np.asarray(W_upper, np.float32)
    wlin = np.asarray(W_lin, np.float32) * EPS
    al = np.asarray(att_lower, np.float32)
    au = np.asarray(att_upper, np.float32)
    # wa4 cols -> fields: [ss_l, ss_u, st_l, st_u]
    wa4 = np.stack([wl @ al[d:], wu @ au[d:], wl @ al[:d], wu @ au[:d]],
                   axis=1).astype(ml_dtypes.bfloat16)   # [d, 4]
    wa42 = np.ascontiguousarray(wa4.reshape(2, 128, 4).transpose(1, 0, 2))

    def w2(w):
        return np.ascontiguousarray(
            w.astype(ml_dtypes.bfloat16).reshape(2, 128, d).transpose(1, 0, 2))

    iota_np = np.broadcast_to(np.arange(128), (128, 128)).astype(
        ml_dtypes.bfloat16)

    common = dict(xs_tab=xs_np, xt2=xt2, wa42=wa42,
                  w_l2=w2(wl), w_u2=w2(wu), w_lin2=w2(wlin),
                  iota_in=iota_np)
    in_maps = []
    for k in range(n_cores):
        m = dict(common)
        xo = np.ascontiguousarray(x_pad[k * own:k * own + ownp].T).astype(
            ml_dtypes.bfloat16)
        m["xot2"] = np.ascontiguousarray(
            xo.reshape(2, 128, ownp).transpose(1, 0, 2))
        for cv, cores in (("l", cores_l), ("u", cores_u)):
            cdk = cores[k]
            m[f"toff_{cv}"] = cdk["toff"]
            m[f"val_{cv}"] = cdk["val"]
        m["toffi_c"] = cores_l[k]["toffi_c"]
        m["idxa_c"] = cores_l[k]["idxa_c"]
        m["idxb_c"] = cores_l[k]["idxb_c"]
        in_maps.append(m)

    dims = dict(n=n, npad=npad, d=d, n_cores=n_cores, own=own, ntiles=ntiles,
                split=split)
    return dims, meta_l, meta_u, in_maps


def build_all(inputs, n_cores=8, split=32768):
    dims, meta_l, meta_u, in_maps = _prepare(**inputs, n_cores=n_cores,
                                             split=split)
    nc = _build_program(dims["n"], dims["npad"], dims["d"], dims["n_cores"],
                        dims["own"], dims["ntiles"], dims["split"],
                        meta_l, meta_u)
    return nc, in_maps, dims


def kernel(**inputs):
    nc, in_maps, dims = build_all(inputs)
    res = run_bass_kernel_spmd(nc, in_maps, list(range(dims["n_cores"])))
    outs = [res.results[k]["out"][:dims["own"]] for k in range(dims["n_cores"])]
    return np.concatenate(outs, axis=0)[:dims["n"]].astype(np.float32)


# revision 5
# speedup vs baseline: 1.5055x; 1.2221x over previous
"""Trainium2 Bass kernel for nn_CANLayer (CAN GNN layer) — v2.

Strategy (8-core SPMD, no collectives):
  - Targets sharded 6272/core (49 tiles of 128, 128-aligned ownership).
    Edges routed to the target-owning core; every softmax segment is local.
  - Phase 1 (scores): node-major matvec on PE (lhsT = x^T blocks, rhs = the
    four projected attention vectors); the ss pair is written into the
    combined gather table xs_tab cols 256:260 (f32 pair packed in bf16
    cols) in one descriptor-heavy DMA, the st pair for own targets into
    st_tab.  No transposes, no zeroing passes.
  - Combined gather row (768B): [x bf16(256) | ss_l,ss_u f32(8B) | pad].
    Gather calls are merged across the two convs per tile (A_l|A_u and
    B_l|B_u index streams share one output buffer), <=1024 indices per
    call (HW SWDGE ring cap).  No st gather at all: per chunk, a PE outer
    product broadcasts the chunk's target offsets, a DVE is_equal against
    a row-index constant builds the transposed one-hot, and a tiny PE
    matmul against the tile's [128,2] st values expands st per edge.
  - Per 128-edge chunk: fused one-hot build maskw[e,t] = (iota==tof)*w with
    w = exp(elu(st+ss)*val) computed in 6 batched ops; then
    U^T[d,t] += matmul(lhsT=xg d-half, rhs=maskw) into per-bank PSUM and
    s[t] += matmul(lhsT=maskw, rhs=ones) (s as a column for per-partition
    scaling later).  Unnormalized U^T copied to bf16 per tile.
  - Convs l/u interleaved per tile; phase 3 fused into the tile loop:
    out = relu(rs_l*(U_l^T.T@W_l) + rs_u*(U_u^T.T@W_u) + x@(W_lin*EPS))
    with rs = 1/max(s,eps) per-partition scalars, batched 7-tile output
    writes.  Whole-conv idx/toff/val metadata loaded in a few mega-DMAs.
"""

import os
import sys
from contextlib import ExitStack

import numpy as np
import ml_dtypes

for _p in ("/opt/trn_rl_repo", "/root/.axon_site/_ro/trn_rl_repo"):
    if os.path.isdir(_p) and _p not in sys.path:
        sys.path.insert(0, _p)

import concourse.bass as bass
import concourse.bacc as bacc
import concourse.tile as tile
from concourse import mybir
from concourse.bass_utils import run_bass_kernel_spmd

BF16 = mybir.dt.bfloat16
FP8 = mybir.dt.float8e4
F32 = mybir.dt.float32
I16 = mybir.dt.int16
ALU = mybir.AluOpType
ACTF = mybir.ActivationFunctionType

EPS = 1.0 + 1e-06
PAD_OFF = 200.0  # tgt_off value for pad edge slots (outside [0,128) window)
ROW = 384        # xs_tab row cols (bf16) = 768 bytes
SS0 = 256        # first score col in xs_tab


def _ceil(a, b):
    return -(-a // b)


# --------------------------------------------------------------------------
# host-side preprocessing
# --------------------------------------------------------------------------

def _prep_conv(indices, values, n, n_cores, own, ntiles, split):
    """Sort/tile/pad one conv's edges.  Returns per-core arrays + baked meta.

    Per tile (128 consecutive owned targets): edges are grouped as
    [A-edges (src < split) | pad | B-edges | pad], each padded up to a
    multiple of 128 with (src=0, val=0, tgt_off=PAD_OFF).  Chunk counts are
    maxed over cores so one SPMD program fits all cores.
    """
    tgt = np.asarray(indices[0], dtype=np.int64).astype(np.int32)
    src = np.asarray(indices[1], dtype=np.int64).astype(np.int32)
    val = np.asarray(values, dtype=np.float32)

    core_of = np.minimum(tgt // own, n_cores - 1)

    per_core = []
    for k in range(n_cores):
        sel = np.nonzero(core_of == k)[0]
        tl = tgt[sel] - k * own
        order = np.argsort(tl, kind="stable")
        sel = sel[order]
        tl = tl[order]
        tile_id = tl >> 7
        bounds = np.searchsorted(tile_id, np.arange(ntiles + 1))
        tiles = []
        for t in range(ntiles):
            e = sel[bounds[t]:bounds[t + 1]]
            isa = src[e] < split
            tiles.append((e[isa], e[~isa]))
        per_core.append(tiles)

    ncha = [max(_ceil(max(len(per_core[k][t][0]) for k in range(n_cores)), 128), 1)
            for t in range(ntiles)]
    nchb = [_ceil(max(len(per_core[k][t][1]) for k in range(n_cores)), 128)
            for t in range(ntiles)]
    ch = [a + b for a, b in zip(ncha, nchb)]
    cht = sum(ch)
    fa = sum(ncha) * 8   # int16 idx cols (128 idx -> 8 cols of 16)
    fb = sum(nchb) * 8

    def wrap_idx(vals16, out, col0):
        # linear idx i -> (partition i%16 [+16*g replicas], col i//16)
        m = len(vals16) // 16
        blk = vals16.reshape(m, 16).T  # [16, m]
        for g in range(8):
            out[g * 16:(g + 1) * 16, col0:col0 + m] = blk

    cores = []
    for k in range(n_cores):
        idxa = np.zeros((128, fa), np.int16)
        idxb = np.zeros((128, max(fb, 1)), np.int16)
        toff = np.full((128, cht), PAD_OFF, np.float32)
        vals = np.zeros((128, cht), np.float32)
        toffi = np.zeros((128, cht * 8), np.int16)
        ca = cb = cc = 0
        for t in range(ntiles):
            ea, eb = per_core[k][t]
            for which, e, nch in (("a", ea, ncha[t]), ("b", eb, nchb[t])):
                nslot = nch * 128
                s = np.zeros(nslot, np.int32)
                s[:len(e)] = src[e] if which == "a" else src[e] - split
                to = np.full(nslot, PAD_OFF, np.float32)
                to[:len(e)] = (tgt[e] - k * own - t * 128).astype(np.float32)
                vv = np.zeros(nslot, np.float32)
                vv[:len(e)] = val[e]
                # chunk-major [p, c] layout: slot i -> (i % 128, i // 128)
                cols = slice(cc, cc + nch)
                toff[:, cols] = to.reshape(nch, 128).T
                vals[:, cols] = vv.reshape(nch, 128).T
                ti = np.zeros(nslot, np.int32)
                ti[:len(e)] = (tgt[e] - k * own - t * 128).astype(np.int32)
                wrap_idx(ti.astype(np.int16), toffi, cc * 8)
                cc += nch
                if which == "a":
                    wrap_idx(s.astype(np.int16), idxa, ca * 8)
                    ca += nch
                else:
                    if nch:
                        wrap_idx(s.astype(np.int16), idxb, cb * 8)
                    cb += nch
        cores.append(dict(idxa=idxa, idxb=idxb, toff=toff, val=vals,
                          toffi=toffi))

    meta = dict(ncha=ncha, nchb=nchb, ch=ch, cht=cht, fa=fa, fb=max(fb, 1),
                ownp=ntiles * 128)
    return meta, cores


# --------------------------------------------------------------------------
# device program
# --------------------------------------------------------------------------

def psum2_tile(p, tag):
    # one full PSUM bank (2 KB/partition) so each matmul accumulation group
    # owns its zero region exclusively
    return p.tile([128, 512], F32, tag=tag, name=f"pb_{tag}")


def _build_program(n, npad, d, n_cores, own, ntiles, split, meta_l, meta_u):
    nc = bacc.Bacc(trn_type="TRN2", target_bir_lowering=False, debug=False,
                   num_devices=n_cores, num_swdge_queues=4)
    ownp = ntiles * 128
    nblk = npad // 128          # 128-node score blocks
    J = 1792                    # phase-1 xt chunk cols (14 blocks)
    assert npad % J == 0
    niter = npad // J

    def din(name, shape, dt):
        return nc.dram_tensor(name, shape, dt, kind="ExternalInput")

    xs_tab = din("xs_tab", [npad, ROW], BF16)
    xt2 = din("xt2", [128, 2, npad], BF16)
    xot2 = din("xot2", [128, 2, ownp], BF16)
    wa42 = din("wa42", [128, 2, 4], BF16)
    w_l2 = din("w_l2", [128, 2, d], BF16)
    w_u2 = din("w_u2", [128, 2, d], BF16)
    w_lin2 = din("w_lin2", [128, 2, d], BF16)
    iota_in = din("iota_in", [128, 128], BF16)
    convs = {}
    for cv, meta in (("l", meta_l), ("u", meta_u)):
        convs[cv] = dict(
            meta=meta,
            toff=din(f"toff_{cv}", [128, meta["cht"]], F32),
            val=din(f"val_{cv}", [128, meta["cht"]], F32),
        )
    tofrow = din("tofrow", [1, meta_l["cht_c"] * 128], BF16)
    idxa_c = din("idxa_c", [128, meta_l["fa_c"]], I16)
    idxb_c = din("idxb_c", [128, meta_l["fb_c"]], I16)
    iotap_in = din("iotap_in", [128, 128], F32)
    st_tab = nc.dram_tensor("st_tab", [ownp, 2], BF16)
    out = nc.dram_tensor("out", [ownp, d], F32, kind="ExternalOutput")

    gmax = 8   # max chunks (x128 idx) per dma_gather call (SWDGE ring)

    def split_gather(out_tile, co, nch, table, idx_sb, io, elem, qiter):
        g0 = 0
        while g0 < nch:
            g = min(gmax, nch - g0)
            nc.gpsimd.dma_gather(
                out_tile[:, co + g0:co + g0 + g, :], table,
                idx_sb[:, (io + g0) * 8:(io + g0 + g) * 8],
                g * 128, g * 128, elem, elem_step=elem, queue_num=qiter[0] % 4)
            qiter[0] += 1
            g0 += g

    with tile.TileContext(nc) as tc:
        with ExitStack() as ctx:
            pool = ctx.enter_context(tc.tile_pool(name="sb", bufs=2))
            mpool = ctx.enter_context(tc.tile_pool(name="meta", bufs=1))
            cpool = ctx.enter_context(tc.tile_pool(name="const", bufs=1))

            iota_t = cpool.tile([128, 128], BF16)
            nc.sync.dma_start(iota_t[:], iota_in[:, :])
            iotap_t = cpool.tile([128, 128], F32)
            nc.sync.dma_start(iotap_t[:], iotap_in[:, :])
            ones_row = cpool.tile([1, 128], BF16)
            nc.vector.memset(ones_row[:], 1.0)
            ones_t = cpool.tile([128, 1], BF16)
            nc.vector.memset(ones_t[:], 1.0)
            wa_sb = cpool.tile([128, 2, 4], BF16)
            nc.sync.dma_start(wa_sb[:], wa42[:, :, :])
            xo_sb = cpool.tile([128, 2, ownp], BF16)
            nc.sync.dma_start(xo_sb[:], xot2[:, :, :])

            # ---- phase 1: scores, node-major on PE -----------------------
            p1cm = tc.tile_pool(name="p1", bufs=2)
            p1 = p1cm.__enter__()
            sc_sb = p1.tile([128, nblk, 4], F32, tag="scsb", bufs=1)
            with tc.tile_pool(name="ps1", bufs=2, space="PSUM") as ps1:
                for it in range(niter):
                    xt_t = p1.tile([128, 2, J], BF16, tag="xt")
                    nc.sync.dma_start(xt_t[:], xt2[:, :, it * J:(it + 1) * J])
                    scp = ps1.tile([128, J // 128, 4], F32, tag="scp", bufs=2)
                    for b in range(J // 128):
                        for h in range(2):
                            nc.tensor.matmul(
                                out=scp[:, b, :],
                                lhsT=xt_t[:, h, b * 128:(b + 1) * 128],
                                rhs=wa_sb[:, h, :],
                                start=(h == 0), stop=(h == 1))
                    nc.vector.tensor_copy(
                        out=sc_sb[:, it * (J // 128):(it + 1) * (J // 128), :],
                        in_=scp[:])
                # ss pair -> xs_tab cols [SS0, SS0+4) as raw f32-pair bytes
                nc.sync.dma_start(
                    xs_tab[0:npad, SS0:SS0 + 4].rearrange(
                        "(c p) f -> p c f", p=128),
                    sc_sb[:, :, 0:2].bitcast(BF16))

                # own-target st pair from resident xot2
                stp = ps1.tile([128, ntiles, 2], F32, tag="stp", bufs=1)
                for b in range(ntiles):
                    for h in range(2):
                        nc.tensor.matmul(
                            out=stp[:, b, :],
                            lhsT=xo_sb[:, h, b * 128:(b + 1) * 128],
                            rhs=wa_sb[:, h, 2:4],
                            start=(h == 0), stop=(h == 1))
                sto_sb = p1.tile([128, ntiles, 2], BF16, tag="sto")
                nc.vector.tensor_copy(out=sto_sb[:], in_=stp[:])
                nc.sync.dma_start(
                    st_tab[0:ownp, 0:2].rearrange("(c p) f -> p c f", p=128),
                    sto_sb[:])

            wsb = {}
            for nm, w_in in (("l", w_l2), ("u", w_u2), ("x", w_lin2)):
                tl = cpool.tile([128, 2, d], BF16, tag=f"w{nm}",
                                name=f"w_{nm}")
                nc.sync.dma_start(tl[:], w_in[:, :, :])
                wsb[nm] = tl

            p1cm.__exit__(None, None, None)
            tc.strict_bb_all_engine_barrier()

            # per-conv metadata mega-loads
            msb = {}
            for cv in ("l", "u"):
                cd = convs[cv]
                meta = cd["meta"]
                d_ = {}
                d_["tof"] = mpool.tile([128, meta["cht"]], F32, tag=f"tof{cv}",
                                       name=f"tof_{cv}")
                nc.sync.dma_start(d_["tof"][:], cd["toff"][:, :])
                d_["val"] = mpool.tile([128, meta["cht"]], F32, tag=f"val{cv}",
                                       name=f"val_{cv}")
                nc.sync.dma_start(d_["val"][:], cd["val"][:, :])
                msb[cv] = d_
            iac_sb = mpool.tile([128, meta_l["fa_c"]], I16, tag="iac")
            nc.sync.dma_start(iac_sb[:], idxa_c[:, :])
            ibc_sb = mpool.tile([128, meta_l["fb_c"]], I16, tag="ibc")
            nc.sync.dma_start(ibc_sb[:], idxb_c[:, :])
            # ---- phase 2+3 interleaved: per tile, conv l then u,
            # then immediate projection + combine + relu ------------------
            qiter = [0]
            ps2 = ctx.enter_context(tc.tile_pool(name="ps2", bufs=1,
                                                 space="PSUM"))
            ps3 = ctx.enter_context(tc.tile_pool(name="ps3", bufs=1,
                                                 space="PSUM"))
            p3 = ctx.enter_context(tc.tile_pool(name="p3", bufs=2))
            offs = {}
            for cv in ("l", "u"):
                meta = convs[cv]["meta"]
                offs[cv] = (np.cumsum([0] + meta["ncha"]),
                            np.cumsum([0] + meta["nchb"]),
                            np.cumsum([0] + meta["ch"]))
            OB = 7  # tiles per output write batch (49 = 7*7)
            ost = None
            offc_c = np.cumsum(
                [0] + [meta_l["ch"][t] + meta_u["ch"][t]
                       for t in range(ntiles)])
            offa_c = np.cumsum(
                [0] + [meta_l["ncha"][t] + meta_u["ncha"][t]
                       for t in range(ntiles)])
            offb_c = np.cumsum(
                [0] + [meta_l["nchb"][t] + meta_u["nchb"][t]
                       for t in range(ntiles)])
            for t in range(ntiles):
                uts = {}
                rss = {}
                ch_l = convs["l"]["meta"]["ch"][t]
                ch_c = ch_l + convs["u"]["meta"]["ch"][t]
                c0_c = offc_c[t]
                # on-chip st expansion: st_e = (transposed one-hot) @ st_tile
                st2 = pool.tile([128, 2], BF16, tag="st2")
                nc.sync.dma_start(st2[:], st_tab[t * 128:(t + 1) * 128, :])
                tr_sb = pool.tile([1, ch_c * 128], BF16, tag="trow")
                nc.sync.dma_start(
                    tr_sb[:], tofrow[0:1, c0_c * 128:(c0_c + ch_c) * 128])
                stps = psum2_tile(ps2, "stps")
                for c in range(ch_c):
                    tb_ps = ps2.tile([128, 512], F32, tag="tbps",
                                     name="pb_tbps", bufs=2)
                    nc.tensor.matmul(
                        out=tb_ps[:, 0:128], lhsT=ones_row[:],
                        rhs=tr_sb[:, c * 128:(c + 1) * 128],
                        start=True, stop=True)
                    mt = pool.tile([128, 128], BF16, tag="mt", bufs=4)
                    nc.vector.tensor_tensor(
                        out=mt[:], in0=iotap_t[:], in1=tb_ps[:, 0:128],
                        op=ALU.is_equal)
                    nc.tensor.matmul(
                        out=stps[:, 2 * c:2 * c + 2], lhsT=mt[:], rhs=st2[:],
                        start=True, stop=True)
                na_l = convs["l"]["meta"]["ncha"][t]
                na_u = convs["u"]["meta"]["ncha"][t]
                nb_l = convs["l"]["meta"]["nchb"][t]
                nb_u = convs["u"]["meta"]["nchb"][t]
                naa = na_l + na_u
                nbb = nb_l + nb_u
                xga = pool.tile([128, naa + nbb, ROW], BF16, tag="xga")
                split_gather(xga, 0, naa, xs_tab[:, :], iac_sb, offa_c[t],
                             ROW, qiter)
                if nbb:
                    split_gather(xga, naa, nbb, xs_tab[split:, :], ibc_sb,
                                 offb_c[t], ROW, qiter)
                for cvi, cv in enumerate(("l", "u")):
                    meta = convs[cv]["meta"]
                    na, nb = meta["ncha"][t], meta["nchb"][t]
                    ch = meta["ch"][t]
                    offa, offb, offc = offs[cv]
                    c0 = offc[t]
                    tof_sb, val_sb = msb[cv]["tof"], msb[cv]["val"]

                    # chunk c of this conv -> column in the combined xga:
                    # layout [A_l | A_u | B_l | B_u]
                    a0 = 0 if cvi == 0 else na_l
                    b0 = naa + (0 if cvi == 0 else nb_l)
                    cols = [a0 + c if c < na else b0 + (c - na)
                            for c in range(ch)]
                    stg0 = 0 if cvi == 0 else ch_l
                    sslo = SS0 + 2 * cvi
                    ss_a = xga[:, a0:a0 + na, sslo:sslo + 2].bitcast(F32)
                    st = stps[:, 2 * stg0:2 * (stg0 + ch)].rearrange(
                        "p (c f) -> p c f", f=2)[:, :, cvi:cvi + 1]
                    tof = tof_sb[:, c0:c0 + ch]
                    vv = val_sb[:, c0:c0 + ch]

                    # w = exp(elu(st+ss) * val); elu(z)=max(z,0)-1+e^min(z,0)
                    z = pool.tile([128, ch], F32, tag=f"z{cv}",
                                  name=f"z_{cv}")
                    nc.vector.tensor_add(out=z[:, 0:na], in0=st[:, 0:na, :],
                                         in1=ss_a)
                    if nb:
                        ss_b = xga[:, b0:b0 + nb,
                                   sslo:sslo + 2].bitcast(F32)
                        nc.vector.tensor_add(out=z[:, na:ch],
                                             in0=st[:, na:ch, :], in1=ss_b)
                    zm = pool.tile([128, ch], F32, tag=f"zm{cv}",
                                   name=f"zm_{cv}")
                    nc.vector.tensor_scalar(out=zm[:], in0=z[:], scalar1=0.0,
                                            scalar2=None, op0=ALU.min)
                    e1 = pool.tile([128, ch], F32, tag=f"e1{cv}",
                                   name=f"e1_{cv}")
                    nc.scalar.activation(e1[:], zm[:], ACTF.Exp)
                    zp = pool.tile([128, ch], F32, tag=f"zp{cv}",
                                   name=f"zp_{cv}")
                    nc.vector.tensor_scalar(out=zp[:], in0=z[:], scalar1=0.0,
                                            scalar2=-1.0, op0=ALU.max,
                                            op1=ALU.add)
                    t1 = pool.tile([128, ch], F32, tag=f"t1{cv}",
                                   name=f"t1_{cv}")
                    nc.vector.tensor_add(out=t1[:], in0=zp[:], in1=e1[:])
                    v = pool.tile([128, ch], F32, tag=f"v{cv}",
                                  name=f"v_{cv}")
                    nc.vector.tensor_mul(out=v[:], in0=t1[:], in1=vv)
                    w = pool.tile([128, ch], F32, tag=f"w{cv}",
                                  name=f"w_{cv}")
                    nc.scalar.activation(w[:], v[:], ACTF.Exp)

                    ut0 = psum2_tile(ps2, "ut0")
                    ut1 = psum2_tile(ps2, "ut1")
                    s_ps = psum2_tile(ps2, "sps")
                    for c in range(ch):
                        mw = pool.tile([128, 128], BF16, tag=f"mw{cv}",
                                       name=f"mw_{cv}", bufs=8)
                        nc.vector.tensor_scalar(
                            out=mw[:], in0=iota_t[:], scalar1=tof[:, c:c + 1],
                            scalar2=w[:, c:c + 1], op0=ALU.is_equal,
                            op1=ALU.mult)
                        xc = cols[c]
                        nc.tensor.matmul(
                            out=ut0[:, 0:128], lhsT=xga[:, xc, 0:128],
                            rhs=mw[:],
                            start=(c == 0), stop=(c == ch - 1))
                        nc.tensor.matmul(
                            out=ut1[:, 0:128], lhsT=xga[:, xc, 128:256],
                            rhs=mw[:],
                            start=(c == 0), stop=(c == ch - 1))
                        nc.tensor.matmul(out=s_ps[:, 0:1], lhsT=mw[:],
                                         rhs=ones_t[:],
                                         start=(c == 0), stop=(c == ch - 1))
                    atk = pool.tile([128, 2, 128], BF16, tag=f"at{cv}",
                                    name=f"at_{cv}")
                    nc.vector.tensor_copy(out=atk[:, 0, :], in_=ut0[:, 0:128])
                    nc.vector.tensor_copy(out=atk[:, 1, :], in_=ut1[:, 0:128])
                    uts[cv] = atk
                    sden = pool.tile([128, 1], F32, tag=f"sden{cv}",
                                     name=f"sden_{cv}")
                    nc.vector.tensor_scalar(out=sden[:], in0=s_ps[:, 0:1],
                                            scalar1=1e-30, scalar2=None,
                                            op0=ALU.max)
                    rsk = pool.tile([128, 1], F32, tag=f"rs{cv}",
                                    name=f"rs_{cv}")
                    nc.vector.reciprocal(rsk[:], sden[:])
                    rss[cv] = rsk

                # ---- phase 3 for tile t --------------------------------
                i = t % OB
                if i == 0:
                    ost = p3.tile([128, OB, d], F32, tag="ost")
                tc0, tc1 = t * 128, (t + 1) * 128
                o123 = ps3.tile([128, 3, d], F32, tag="o123")
                for j, (atk, wk) in enumerate(
                        ((uts["l"], "l"), (uts["u"], "u"), (None, "x"))):
                    for h in range(2):
                        lhs = (atk[:, h, :] if atk is not None
                               else xo_sb[:, h, tc0:tc1])
                        nc.tensor.matmul(out=o123[:, j, :], lhsT=lhs,
                                         rhs=wsb[wk][:, h, :],
                                         start=(h == 0), stop=(h == 1))
                a1 = p3.tile([128, d], F32, tag="a1")
                nc.vector.tensor_scalar(out=a1[:], in0=o123[:, 0, :],
                                        scalar1=rss["l"][:], scalar2=None,
                                        op0=ALU.mult)
                a2 = p3.tile([128, d], F32, tag="a2")
                nc.vector.tensor_scalar(out=a2[:], in0=o123[:, 1, :],
                                        scalar1=rss["u"][:], scalar2=None,
                                        op0=ALU.mult)
                a3 = p3.tile([128, d], F32, tag="a1")
                nc.vector.tensor_add(out=a3[:], in0=a1[:], in1=a2[:])
                a4 = p3.tile([128, d], F32, tag="a2")
                nc.vector.tensor_add(out=a4[:], in0=a3[:], in1=o123[:, 2, :])
                nc.scalar.activation(ost[:, i, :], a4[:], ACTF.Relu)
                if i == OB - 1 or t == ntiles - 1:
                    t0 = t - i
                    nc.sync.dma_start(
                        out[t0 * 128:(t + 1) * 128, :].rearrange(
                            "(b p) f -> p b f", p=128),
                        ost[:, 0:i + 1, :])
    import re as _re
    for blk in nc.m.functions[0].blocks:
        for inst in blk.instructions:
            if isinstance(inst, mybir.InstDMAGatherAnt):
                lane = None
                si = inst.sync_info
                ups = si.on_update if si is not None else []
                for u in ups:
                    m = _re.search(r"DMASW(\d+)", str(u.ant_name))
                    if m:
                        lane = int(m.group(1))
                        break
                if lane is not None:
                    inst.queue_num = lane % 4
    nc.finalize()
    return nc


# --------------------------------------------------------------------------
# top level
# --------------------------------------------------------------------------

def _prepare(x, lower_indices, lower_values, upper_indices, upper_values,
             W_lower, att_lower, W_upper, att_upper, W_lin,
             n_cores=8, split=32768):
    n, d = x.shape
    ntiles = _ceil(_ceil(n, n_cores), 128)
    own = ntiles * 128          # 128-aligned ownership (6272)
    ownp = own
    npad = _ceil(max(n_cores * ownp, n), 512) * 512

    meta_l, cores_l = _prep_conv(lower_indices, lower_values, n, n_cores, own,
                                 ntiles, split)
    meta_u, cores_u = _prep_conv(upper_indices, upper_values, n, n_cores, own,
                                 ntiles, split)

    # combined st-gather indices: per tile, conv-l chunks then conv-u chunks
    offc_l = np.cumsum([0] + meta_l["ch"])
    offc_u = np.cumsum([0] + meta_u["ch"])
    cht_c = meta_l["cht"] + meta_u["cht"]
    # combined xs-gather indices: per tile, [A_l | A_u] and [B_l | B_u]
    offa_l = np.cumsum([0] + meta_l["ncha"])
    offa_u = np.cumsum([0] + meta_u["ncha"])
    offb_l = np.cumsum([0] + meta_l["nchb"])
    offb_u = np.cumsum([0] + meta_u["nchb"])
    fa_c = (sum(meta_l["ncha"]) + sum(meta_u["ncha"])) * 8
    fb_c = max((sum(meta_l["nchb"]) + sum(meta_u["nchb"])) * 8, 8)
    for k in range(n_cores):
        tic = np.zeros((128, cht_c * 8), np.int16)
        iac = np.zeros((128, fa_c), np.int16)
        ibc = np.zeros((128, fb_c), np.int16)
        col = ca = cb = 0
        for t in range(ntiles):
            for cores, offc, offa, offb, meta in (
                    (cores_l, offc_l, offa_l, offb_l, meta_l),
                    (cores_u, offc_u, offa_u, offb_u, meta_u)):
                chv = meta["ch"][t]
                tic[:, col * 8:(col + chv) * 8] = \
                    cores[k]["toffi"][:, offc[t] * 8:(offc[t] + chv) * 8]
                col += chv
            for cores, offa, meta in ((cores_l, offa_l, meta_l),
                                      (cores_u, offa_u, meta_u)):
                nav = meta["ncha"][t]
                iac[:, ca * 8:(ca + nav) * 8] = \
                    cores[k]["idxa"][:, offa[t] * 8:(offa[t] + nav) * 8]
                ca += nav
            for cores, offb, meta in ((cores_l, offb_l, meta_l),
                                      (cores_u, offb_u, meta_u)):
                nbv = meta["nchb"][t]
                if nbv:
                    ibc[:, cb * 8:(cb + nbv) * 8] = \
                        cores[k]["idxb"][:, offb[t] * 8:(offb[t] + nbv) * 8]
                cb += nbv
        cores_l[k]["toffi_c"] = tic
        cores_l[k]["idxa_c"] = iac
        cores_l[k]["idxb_c"] = ibc
        # toff values as rows (partition 0): per tile, l-chunks then u-chunks,
        # chunk-major 128 edges each
        tr = np.zeros((1, cht_c * 128), ml_dtypes.bfloat16)
        col = 0
        for t in range(ntiles):
            for cores, offc, meta in ((cores_l, offc_l, meta_l),
                                      (cores_u, offc_u, meta_u)):
                chv = meta["ch"][t]
                blk = cores[k]["toff"][:, offc[t]:offc[t] + chv]  # [128, chv]
                tr[0, col * 128:(col + chv) * 128] = \
                    blk.T.reshape(-1).astype(ml_dtypes.bfloat16)
                col += chv
        cores_l[k]["tofrow"] = tr
    meta_l["cht_c"] = cht_c
    meta_l["fa_c"] = fa_c
    meta_l["fb_c"] = fb_c

    xf = np.asarray(x, np.float32)
    x_pad = np.zeros((npad, d), np.float32)
    x_pad[:n] = xf
    xs_np = np.zeros((npad, ROW), ml_dtypes.bfloat16)
    xs_np[:, :d] = x_pad.astype(ml_dtypes.bfloat16)
    xt_b = np.ascontiguousarray(x_pad.T).astype(ml_dtypes.bfloat16)  # [d, npad]
    xt2 = np.ascontiguousarray(
        xt_b.reshape(2, 128, npad).transpose(1, 0, 2))  # [128, 2, npad]

    wl = np.asarray(W_lower, np.float32)
    wu = np.asarray(W_upper, np.float32)
    wlin = np.asarray(W_lin, np.float32) * EPS
    al = np.asarray(att_lower, np.float32)
    au = np.asarray(att_upper, np.float32)
    # wa4 cols -> fields: [ss_l, ss_u, st_l, st_u]
    wa4 = np.stack([wl @ al[d:], wu @ au[d:], wl @ al[:d], wu @ au[:d]],
                   axis=1).astype(ml_dtypes.bfloat16)   # [d, 4]
    wa42 = np.ascontiguousarray(wa4.reshape(2, 128, 4).transpose(1, 0, 2))

    def w2(w):
        return np.ascontiguousarray(
            w.astype(ml_dtypes.bfloat16).reshape(2, 128, d).transpose(1, 0, 2))

    iota_np = np.broadcast_to(np.arange(128), (128, 128)).astype(
        ml_dtypes.bfloat16)
    iotap_np = np.broadcast_to(np.arange(128)[:, None],
                               (128, 128)).astype(np.float32)

    common = dict(xs_tab=xs_np, xt2=xt2, wa42=wa42,
                  w_l2=w2(wl), w_u2=w2(wu), w_lin2=w2(wlin),
                  iota_in=iota_np, iotap_in=iotap_np)
    in_maps = []
    for k in range(n_cores):
        m = dict(common)
        xo = np.ascontiguousarray(x_pad[k * own:k * own + ownp].T).astype(
            ml_dtypes.bfloat16)
        m["xot2"] = np.ascontiguousarray(
            xo.reshape(2, 128, ownp).transpose(1, 0, 2))
        for cv, cores in (("l", cores_l), ("u", cores_u)):
            cdk = cores[k]
            m[f"toff_{cv}"] = cdk["toff"]
            m[f"val_{cv}"] = cdk["val"]
        m["tofrow"] = cores_l[k]["tofrow"]
        m["idxa_c"] = cores_l[k]["idxa_c"]
        m["idxb_c"] = cores_l[k]["idxb_c"]
        in_maps.append(m)

    dims = dict(n=n, npad=npad, d=d, n_cores=n_cores, own=own, ntiles=ntiles,
                split=split)
    return dims, meta_l, meta_u, in_maps


def build_all(inputs, n_cores=8, split=32768):
    dims, meta_l, meta_u, in_maps = _prepare(**inputs, n_cores=n_cores,
                                             split=split)
    nc = _build_program(dims["n"], dims["npad"], dims["d"], dims["n_cores"],
                        dims["own"], dims["ntiles"], dims["split"],
                        meta_l, meta_u)
    return nc, in_maps, dims


def kernel(**inputs):
    nc, in_maps, dims = build_all(inputs)
    res = run_bass_kernel_spmd(nc, in_maps, list(range(dims["n_cores"])))
    outs = [res.results[k]["out"][:dims["own"]] for k in range(dims["n_cores"])]
    return np.concatenate(outs, axis=0)[:dims["n"]].astype(np.float32)


# revision 6
# speedup vs baseline: 1.5736x; 1.0452x over previous
"""Trainium2 Bass kernel for nn_CANLayer (CAN GNN layer) — v2.

Strategy (8-core SPMD, no collectives):
  - Targets sharded 6272/core (49 tiles of 128, 128-aligned ownership).
    Edges routed to the target-owning core; every softmax segment is local.
  - Phase 1 (scores): node-major matvec on PE (lhsT = x^T blocks, rhs = the
    four projected attention vectors); the ss pair is written into the
    combined gather table xs_tab cols 256:260 (f32 pair packed in bf16
    cols) in one descriptor-heavy DMA, the st pair for own targets into
    st_tab.  No transposes, no zeroing passes.
  - Combined gather row (768B): [x bf16(256) | ss_l,ss_u f32(8B) | pad].
    Gather calls are merged across the two convs per tile (A_l|A_u and
    B_l|B_u index streams share one output buffer), <=1024 indices per
    call (HW SWDGE ring cap).  No st gather at all: batched 4 chunks at
    a time, a PE outer product broadcasts the chunks' target offsets, a
    DVE is_equal against a replicated row-index constant builds the
    transposed one-hots, and tiny PE matmuls against the tile's [128,2]
    st values expand st per edge.
  - Per 128-edge chunk: fused one-hot build maskw[e,t] = (iota==tof)*w with
    w = exp(elu(st+ss)*val) computed in 6 batched ops; then
    U^T[d,t] += matmul(lhsT=xg d-half, rhs=maskw) into per-bank PSUM and
    s[t] += matmul(lhsT=maskw, rhs=ones) (s as a column for per-partition
    scaling later).  Unnormalized U^T copied to bf16 per tile.
  - Convs l/u interleaved per tile; phase 3 fused into the tile loop:
    out = relu(rs_l*(U_l^T.T@W_l) + rs_u*(U_u^T.T@W_u) + x@(W_lin*EPS))
    with rs = 1/max(s,eps) per-partition scalars, batched 7-tile output
    writes.  Whole-conv idx/toff/val metadata loaded in a few mega-DMAs.
"""

import os
import sys
from contextlib import ExitStack

import numpy as np
import ml_dtypes

for _p in ("/opt/trn_rl_repo", "/root/.axon_site/_ro/trn_rl_repo"):
    if os.path.isdir(_p) and _p not in sys.path:
        sys.path.insert(0, _p)

import concourse.bass as bass
import concourse.bacc as bacc
import concourse.tile as tile
from concourse import mybir
from concourse.bass_utils import run_bass_kernel_spmd

BF16 = mybir.dt.bfloat16
FP8 = mybir.dt.float8e4
F32 = mybir.dt.float32
I16 = mybir.dt.int16
ALU = mybir.AluOpType
ACTF = mybir.ActivationFunctionType

EPS = 1.0 + 1e-06
PAD_OFF = 200.0  # tgt_off value for pad edge slots (outside [0,128) window)
ROW = 384        # xs_tab row cols (bf16) = 768 bytes
SS0 = 256        # first score col in xs_tab


def _ceil(a, b):
    return -(-a // b)


# --------------------------------------------------------------------------
# host-side preprocessing
# --------------------------------------------------------------------------

def _prep_conv(indices, values, n, n_cores, own, ntiles, split):
    """Sort/tile/pad one conv's edges.  Returns per-core arrays + baked meta.

    Per tile (128 consecutive owned targets): edges are grouped as
    [A-edges (src < split) | pad | B-edges | pad], each padded up to a
    multiple of 128 with (src=0, val=0, tgt_off=PAD_OFF).  Chunk counts are
    maxed over cores so one SPMD program fits all cores.
    """
    tgt = np.asarray(indices[0], dtype=np.int64).astype(np.int32)
    src = np.asarray(indices[1], dtype=np.int64).astype(np.int32)
    val = np.asarray(values, dtype=np.float32)

    core_of = np.minimum(tgt // own, n_cores - 1)

    per_core = []
    for k in range(n_cores):
        sel = np.nonzero(core_of == k)[0]
        tl = tgt[sel] - k * own
        order = np.argsort(tl, kind="stable")
        sel = sel[order]
        tl = tl[order]
        tile_id = tl >> 7
        bounds = np.searchsorted(tile_id, np.arange(ntiles + 1))
        tiles = []
        for t in range(ntiles):
            e = sel[bounds[t]:bounds[t + 1]]
            isa = src[e] < split
            tiles.append((e[isa], e[~isa]))
        per_core.append(tiles)

    ncha = [max(_ceil(max(len(per_core[k][t][0]) for k in range(n_cores)), 128), 1)
            for t in range(ntiles)]
    nchb = [_ceil(max(len(per_core[k][t][1]) for k in range(n_cores)), 128)
            for t in range(ntiles)]
    ch = [a + b for a, b in zip(ncha, nchb)]
    cht = sum(ch)
    fa = sum(ncha) * 8   # int16 idx cols (128 idx -> 8 cols of 16)
    fb = sum(nchb) * 8

    def wrap_idx(vals16, out, col0):
        # linear idx i -> (partition i%16 [+16*g replicas], col i//16)
        m = len(vals16) // 16
        blk = vals16.reshape(m, 16).T  # [16, m]
        for g in range(8):
            out[g * 16:(g + 1) * 16, col0:col0 + m] = blk

    cores = []
    for k in range(n_cores):
        idxa = np.zeros((128, fa), np.int16)
        idxb = np.zeros((128, max(fb, 1)), np.int16)
        toff = np.full((128, cht), PAD_OFF, np.float32)
        vals = np.zeros((128, cht), np.float32)
        toffi = np.zeros((128, cht * 8), np.int16)
        ca = cb = cc = 0
        for t in range(ntiles):
            ea, eb = per_core[k][t]
            for which, e, nch in (("a", ea, ncha[t]), ("b", eb, nchb[t])):
                nslot = nch * 128
                s = np.zeros(nslot, np.int32)
                s[:len(e)] = src[e] if which == "a" else src[e] - split
                to = np.full(nslot, PAD_OFF, np.float32)
                to[:len(e)] = (tgt[e] - k * own - t * 128).astype(np.float32)
                vv = np.zeros(nslot, np.float32)
                vv[:len(e)] = val[e]
                # chunk-major [p, c] layout: slot i -> (i % 128, i // 128)
                cols = slice(cc, cc + nch)
                toff[:, cols] = to.reshape(nch, 128).T
                vals[:, cols] = vv.reshape(nch, 128).T
                ti = np.zeros(nslot, np.int32)
                ti[:len(e)] = (tgt[e] - k * own - t * 128).astype(np.int32)
                wrap_idx(ti.astype(np.int16), toffi, cc * 8)
                cc += nch
                if which == "a":
                    wrap_idx(s.astype(np.int16), idxa, ca * 8)
                    ca += nch
                else:
                    if nch:
                        wrap_idx(s.astype(np.int16), idxb, cb * 8)
                    cb += nch
        cores.append(dict(idxa=idxa, idxb=idxb, toff=toff, val=vals,
                          toffi=toffi))

    meta = dict(ncha=ncha, nchb=nchb, ch=ch, cht=cht, fa=fa, fb=max(fb, 1),
                ownp=ntiles * 128)
    return meta, cores


# --------------------------------------------------------------------------
# device program
# --------------------------------------------------------------------------

def psum2_tile(p, tag):
    # one full PSUM bank (2 KB/partition) so each matmul accumulation group
    # owns its zero region exclusively
    return p.tile([128, 512], F32, tag=tag, name=f"pb_{tag}")


def _build_program(n, npad, d, n_cores, own, ntiles, split, meta_l, meta_u):
    nc = bacc.Bacc(trn_type="TRN2", target_bir_lowering=False, debug=False,
                   num_devices=n_cores, num_swdge_queues=4)
    ownp = ntiles * 128
    nblk = npad // 128          # 128-node score blocks
    J = 1792                    # phase-1 xt chunk cols (14 blocks)
    assert npad % J == 0
    niter = npad // J

    def din(name, shape, dt):
        return nc.dram_tensor(name, shape, dt, kind="ExternalInput")

    xs_tab = din("xs_tab", [npad, ROW], BF16)
    xt2 = din("xt2", [128, 2, npad], BF16)
    xot2 = din("xot2", [128, 2, ownp], BF16)
    wa42 = din("wa42", [128, 2, 4], BF16)
    w_l2 = din("w_l2", [128, 2, d], BF16)
    w_u2 = din("w_u2", [128, 2, d], BF16)
    w_lin2 = din("w_lin2", [128, 2, d], BF16)
    iota_in = din("iota_in", [128, 128], BF16)
    convs = {}
    for cv, meta in (("l", meta_l), ("u", meta_u)):
        convs[cv] = dict(
            meta=meta,
            toff=din(f"toff_{cv}", [128, meta["cht"]], F32),
            val=din(f"val_{cv}", [128, meta["cht"]], F32),
        )
    tofrow = din("tofrow", [1, meta_l["cht_c"] * 128], BF16)
    idxa_c = din("idxa_c", [128, meta_l["fa_c"]], I16)
    idxb_c = din("idxb_c", [128, meta_l["fb_c"]], I16)
    iotap_in = din("iotap_in", [128, 4, 128], F32)
    st_tab = nc.dram_tensor("st_tab", [ownp, 2], BF16)
    out = nc.dram_tensor("out", [ownp, d], F32, kind="ExternalOutput")

    gmax = 8   # max chunks (x128 idx) per dma_gather call (SWDGE ring)

    def split_gather(out_tile, co, nch, table, idx_sb, io, elem, qiter):
        g0 = 0
        while g0 < nch:
            g = min(gmax, nch - g0)
            nc.gpsimd.dma_gather(
                out_tile[:, co + g0:co + g0 + g, :], table,
                idx_sb[:, (io + g0) * 8:(io + g0 + g) * 8],
                g * 128, g * 128, elem, elem_step=elem, queue_num=qiter[0] % 4)
            qiter[0] += 1
            g0 += g

    with tile.TileContext(nc) as tc:
        with ExitStack() as ctx:
            pool = ctx.enter_context(tc.tile_pool(name="sb", bufs=2))
            mpool = ctx.enter_context(tc.tile_pool(name="meta", bufs=1))
            cpool = ctx.enter_context(tc.tile_pool(name="const", bufs=1))

            iota_t = cpool.tile([128, 128], BF16)
            nc.sync.dma_start(iota_t[:], iota_in[:, :])
            iotap_t = cpool.tile([128, 4, 128], F32)
            nc.sync.dma_start(iotap_t[:], iotap_in[:, :, :])
            ones_row = cpool.tile([1, 128], BF16)
            nc.vector.memset(ones_row[:], 1.0)
            ones_t = cpool.tile([128, 1], BF16)
            nc.vector.memset(ones_t[:], 1.0)
            wa_sb = cpool.tile([128, 2, 4], BF16)
            nc.sync.dma_start(wa_sb[:], wa42[:, :, :])
            xo_sb = cpool.tile([128, 2, ownp], BF16)
            nc.sync.dma_start(xo_sb[:], xot2[:, :, :])

            # ---- phase 1: scores, node-major on PE -----------------------
            p1cm = tc.tile_pool(name="p1", bufs=2)
            p1 = p1cm.__enter__()
            sc_sb = p1.tile([128, nblk, 4], F32, tag="scsb", bufs=1)
            with tc.tile_pool(name="ps1", bufs=2, space="PSUM") as ps1:
                for it in range(niter):
                    xt_t = p1.tile([128, 2, J], BF16, tag="xt")
                    nc.sync.dma_start(xt_t[:], xt2[:, :, it * J:(it + 1) * J])
                    scp = ps1.tile([128, J // 128, 4], F32, tag="scp", bufs=2)
                    for b in range(J // 128):
                        for h in range(2):
                            nc.tensor.matmul(
                                out=scp[:, b, :],
                                lhsT=xt_t[:, h, b * 128:(b + 1) * 128],
                                rhs=wa_sb[:, h, :],
                                start=(h == 0), stop=(h == 1))
                    nc.vector.tensor_copy(
                        out=sc_sb[:, it * (J // 128):(it + 1) * (J // 128), :],
                        in_=scp[:])
                # ss pair -> xs_tab cols [SS0, SS0+4) as raw f32-pair bytes
                nc.sync.dma_start(
                    xs_tab[0:npad, SS0:SS0 + 4].rearrange(
                        "(c p) f -> p c f", p=128),
                    sc_sb[:, :, 0:2].bitcast(BF16))

                # own-target st pair from resident xot2
                stp = ps1.tile([128, ntiles, 2], F32, tag="stp", bufs=1)
                for b in range(ntiles):
                    for h in range(2):
                        nc.tensor.matmul(
                            out=stp[:, b, :],
                            lhsT=xo_sb[:, h, b * 128:(b + 1) * 128],
                            rhs=wa_sb[:, h, 2:4],
                            start=(h == 0), stop=(h == 1))
                sto_sb = p1.tile([128, ntiles, 2], BF16, tag="sto")
                nc.vector.tensor_copy(out=sto_sb[:], in_=stp[:])
                nc.sync.dma_start(
                    st_tab[0:ownp, 0:2].rearrange("(c p) f -> p c f", p=128),
                    sto_sb[:])

            wsb = {}
            for nm, w_in in (("l", w_l2), ("u", w_u2), ("x", w_lin2)):
                tl = cpool.tile([128, 2, d], BF16, tag=f"w{nm}",
                                name=f"w_{nm}")
                nc.sync.dma_start(tl[:], w_in[:, :, :])
                wsb[nm] = tl

            p1cm.__exit__(None, None, None)
            tc.strict_bb_all_engine_barrier()

            # per-conv metadata mega-loads
            msb = {}
            for cv in ("l", "u"):
                cd = convs[cv]
                meta = cd["meta"]
                d_ = {}
                d_["tof"] = mpool.tile([128, meta["cht"]], F32, tag=f"tof{cv}",
                                       name=f"tof_{cv}")
                nc.sync.dma_start(d_["tof"][:], cd["toff"][:, :])
                d_["val"] = mpool.tile([128, meta["cht"]], F32, tag=f"val{cv}",
                                       name=f"val_{cv}")
                nc.sync.dma_start(d_["val"][:], cd["val"][:, :])
                msb[cv] = d_
            iac_sb = mpool.tile([128, meta_l["fa_c"]], I16, tag="iac")
            nc.sync.dma_start(iac_sb[:], idxa_c[:, :])
            ibc_sb = mpool.tile([128, meta_l["fb_c"]], I16, tag="ibc")
            nc.sync.dma_start(ibc_sb[:], idxb_c[:, :])
            # ---- phase 2+3 interleaved: per tile, conv l then u,
            # then immediate projection + combine + relu ------------------
            qiter = [0]
            ps2 = ctx.enter_context(tc.tile_pool(name="ps2", bufs=1,
                                                 space="PSUM"))
            ps3 = ctx.enter_context(tc.tile_pool(name="ps3", bufs=1,
                                                 space="PSUM"))
            p3 = ctx.enter_context(tc.tile_pool(name="p3", bufs=2))
            offs = {}
            for cv in ("l", "u"):
                meta = convs[cv]["meta"]
                offs[cv] = (np.cumsum([0] + meta["ncha"]),
                            np.cumsum([0] + meta["nchb"]),
                            np.cumsum([0] + meta["ch"]))
            OB = 7  # tiles per output write batch (49 = 7*7)
            ost = None
            offc_c = np.cumsum(
                [0] + [meta_l["ch"][t] + meta_u["ch"][t]
                       for t in range(ntiles)])
            offa_c = np.cumsum(
                [0] + [meta_l["ncha"][t] + meta_u["ncha"][t]
                       for t in range(ntiles)])
            offb_c = np.cumsum(
                [0] + [meta_l["nchb"][t] + meta_u["nchb"][t]
                       for t in range(ntiles)])
            for t in range(ntiles):
                uts = {}
                rss = {}
                ch_l = convs["l"]["meta"]["ch"][t]
                ch_c = ch_l + convs["u"]["meta"]["ch"][t]
                c0_c = offc_c[t]
                # on-chip st expansion: st_e = (transposed one-hot) @ st_tile
                st2 = pool.tile([128, 2], BF16, tag="st2")
                nc.sync.dma_start(st2[:], st_tab[t * 128:(t + 1) * 128, :])
                tr_sb = pool.tile([1, ch_c * 128], BF16, tag="trow")
                nc.sync.dma_start(
                    tr_sb[:], tofrow[0:1, c0_c * 128:(c0_c + ch_c) * 128])
                stps = psum2_tile(ps2, "stps")
                for g0 in range(0, ch_c, 4):
                    gn = min(4, ch_c - g0)
                    tb_ps = ps2.tile([128, 512], F32, tag="tbps",
                                     name="pb_tbps", bufs=2)
                    nc.tensor.matmul(
                        out=tb_ps[:, 0:gn * 128], lhsT=ones_row[:],
                        rhs=tr_sb[:, g0 * 128:(g0 + gn) * 128],
                        start=True, stop=True)
                    mt = pool.tile([128, 4, 128], BF16, tag="mt", bufs=4)
                    nc.vector.tensor_tensor(
                        out=mt[:, 0:gn, :], in0=iotap_t[:, 0:gn, :],
                        in1=tb_ps[:, 0:gn * 128].rearrange(
                            "p (c f) -> p c f", f=128),
                        op=ALU.is_equal)
                    for i in range(gn):
                        c = g0 + i
                        nc.tensor.matmul(
                            out=stps[:, 2 * c:2 * c + 2], lhsT=mt[:, i, :],
                            rhs=st2[:], start=True, stop=True)
                na_l = convs["l"]["meta"]["ncha"][t]
                na_u = convs["u"]["meta"]["ncha"][t]
                nb_l = convs["l"]["meta"]["nchb"][t]
                nb_u = convs["u"]["meta"]["nchb"][t]
                naa = na_l + na_u
                nbb = nb_l + nb_u
                xga = pool.tile([128, naa + nbb, ROW], BF16, tag="xga")
                split_gather(xga, 0, naa, xs_tab[:, :], iac_sb, offa_c[t],
                             ROW, qiter)
                if nbb:
                    split_gather(xga, naa, nbb, xs_tab[split:, :], ibc_sb,
                                 offb_c[t], ROW, qiter)
                for cvi, cv in enumerate(("l", "u")):
                    meta = convs[cv]["meta"]
                    na, nb = meta["ncha"][t], meta["nchb"][t]
                    ch = meta["ch"][t]
                    offa, offb, offc = offs[cv]
                    c0 = offc[t]
                    tof_sb, val_sb = msb[cv]["tof"], msb[cv]["val"]

                    # chunk c of this conv -> column in the combined xga:
                    # layout [A_l | A_u | B_l | B_u]
                    a0 = 0 if cvi == 0 else na_l
                    b0 = naa + (0 if cvi == 0 else nb_l)
                    cols = [a0 + c if c < na else b0 + (c - na)
                            for c in range(ch)]
                    stg0 = 0 if cvi == 0 else ch_l
                    sslo = SS0 + 2 * cvi
                    ss_a = xga[:, a0:a0 + na, sslo:sslo + 2].bitcast(F32)
                    st = stps[:, 2 * stg0:2 * (stg0 + ch)].rearrange(
                        "p (c f) -> p c f", f=2)[:, :, cvi:cvi + 1]
                    tof = tof_sb[:, c0:c0 + ch]
                    vv = val_sb[:, c0:c0 + ch]

                    # w = exp(elu(st+ss) * val); elu(z)=max(z,0)-1+e^min(z,0)
                    z = pool.tile([128, ch], F32, tag=f"z{cv}",
                                  name=f"z_{cv}")
                    nc.vector.tensor_add(out=z[:, 0:na], in0=st[:, 0:na, :],
                                         in1=ss_a)
                    if nb:
                        ss_b = xga[:, b0:b0 + nb,
                                   sslo:sslo + 2].bitcast(F32)
                        nc.vector.tensor_add(out=z[:, na:ch],
                                             in0=st[:, na:ch, :], in1=ss_b)
                    zm = pool.tile([128, ch], F32, tag=f"zm{cv}",
                                   name=f"zm_{cv}")
                    nc.vector.tensor_scalar(out=zm[:], in0=z[:], scalar1=0.0,
                                            scalar2=None, op0=ALU.min)
                    e1 = pool.tile([128, ch], F32, tag=f"e1{cv}",
                                   name=f"e1_{cv}")
                    nc.scalar.activation(e1[:], zm[:], ACTF.Exp)
                    zp = pool.tile([128, ch], F32, tag=f"zp{cv}",
                                   name=f"zp_{cv}")
                    nc.vector.tensor_scalar(out=zp[:], in0=z[:], scalar1=0.0,
                                            scalar2=-1.0, op0=ALU.max,
                                            op1=ALU.add)
                    t1 = pool.tile([128, ch], F32, tag=f"t1{cv}",
                                   name=f"t1_{cv}")
                    nc.vector.tensor_add(out=t1[:], in0=zp[:], in1=e1[:])
                    v = pool.tile([128, ch], F32, tag=f"v{cv}",
                                  name=f"v_{cv}")
                    nc.vector.tensor_mul(out=v[:], in0=t1[:], in1=vv)
                    w = pool.tile([128, ch], F32, tag=f"w{cv}",
                                  name=f"w_{cv}")
                    nc.scalar.activation(w[:], v[:], ACTF.Exp)

                    ut0 = psum2_tile(ps2, "ut0")
                    ut1 = psum2_tile(ps2, "ut1")
                    s_ps = psum2_tile(ps2, "sps")
                    for c in range(ch):
                        mw = pool.tile([128, 128], BF16, tag=f"mw{cv}",
                                       name=f"mw_{cv}", bufs=8)
                        nc.vector.tensor_scalar(
                            out=mw[:], in0=iota_t[:], scalar1=tof[:, c:c + 1],
                            scalar2=w[:, c:c + 1], op0=ALU.is_equal,
                            op1=ALU.mult)
                        xc = cols[c]
                        nc.tensor.matmul(
                            out=ut0[:, 0:128], lhsT=xga[:, xc, 0:128],
                            rhs=mw[:],
                            start=(c == 0), stop=(c == ch - 1))
                        nc.tensor.matmul(
                            out=ut1[:, 0:128], lhsT=xga[:, xc, 128:256],
                            rhs=mw[:],
                            start=(c == 0), stop=(c == ch - 1))
                        nc.tensor.matmul(out=s_ps[:, 0:1], lhsT=mw[:],
                                         rhs=ones_t[:],
                                         start=(c == 0), stop=(c == ch - 1))
                    atk = pool.tile([128, 2, 128], BF16, tag=f"at{cv}",
                                    name=f"at_{cv}")
                    nc.vector.tensor_copy(out=atk[:, 0, :], in_=ut0[:, 0:128])
                    nc.vector.tensor_copy(out=atk[:, 1, :], in_=ut1[:, 0:128])
                    uts[cv] = atk
                    sden = pool.tile([128, 1], F32, tag=f"sden{cv}",
                                     name=f"sden_{cv}")
                    nc.vector.tensor_scalar(out=sden[:], in0=s_ps[:, 0:1],
                                            scalar1=1e-30, scalar2=None,
                                            op0=ALU.max)
                    rsk = pool.tile([128, 1], F32, tag=f"rs{cv}",
                                    name=f"rs_{cv}")
                    nc.vector.reciprocal(rsk[:], sden[:])
                    rss[cv] = rsk

                # ---- phase 3 for tile t --------------------------------
                i = t % OB
                if i == 0:
                    ost = p3.tile([128, OB, d], F32, tag="ost")
                tc0, tc1 = t * 128, (t + 1) * 128
                o123 = ps3.tile([128, 3, d], F32, tag="o123")
                for j, (atk, wk) in enumerate(
                        ((uts["l"], "l"), (uts["u"], "u"), (None, "x"))):
                    for h in range(2):
                        lhs = (atk[:, h, :] if atk is not None
                               else xo_sb[:, h, tc0:tc1])
                        nc.tensor.matmul(out=o123[:, j, :], lhsT=lhs,
                                         rhs=wsb[wk][:, h, :],
                                         start=(h == 0), stop=(h == 1))
                a1 = p3.tile([128, d], F32, tag="a1")
                nc.vector.tensor_scalar(out=a1[:], in0=o123[:, 0, :],
                                        scalar1=rss["l"][:], scalar2=None,
                                        op0=ALU.mult)
                a2 = p3.tile([128, d], F32, tag="a2")
                nc.vector.tensor_scalar(out=a2[:], in0=o123[:, 1, :],
                                        scalar1=rss["u"][:], scalar2=None,
                                        op0=ALU.mult)
                a3 = p3.tile([128, d], F32, tag="a1")
                nc.vector.tensor_add(out=a3[:], in0=a1[:], in1=a2[:])
                a4 = p3.tile([128, d], F32, tag="a2")
                nc.vector.tensor_add(out=a4[:], in0=a3[:], in1=o123[:, 2, :])
                nc.scalar.activation(ost[:, i, :], a4[:], ACTF.Relu)
                if i == OB - 1 or t == ntiles - 1:
                    t0 = t - i
                    nc.sync.dma_start(
                        out[t0 * 128:(t + 1) * 128, :].rearrange(
                            "(b p) f -> p b f", p=128),
                        ost[:, 0:i + 1, :])
    import re as _re
    for blk in nc.m.functions[0].blocks:
        for inst in blk.instructions:
            if isinstance(inst, mybir.InstDMAGatherAnt):
                lane = None
                si = inst.sync_info
                ups = si.on_update if si is not None else []
                for u in ups:
                    m = _re.search(r"DMASW(\d+)", str(u.ant_name))
                    if m:
                        lane = int(m.group(1))
                        break
                if lane is not None:
                    inst.queue_num = lane % 4
    nc.finalize()
    return nc


# --------------------------------------------------------------------------
# top level
# --------------------------------------------------------------------------

def _prepare(x, lower_indices, lower_values, upper_indices, upper_values,
             W_lower, att_lower, W_upper, att_upper, W_lin,
             n_cores=8, split=32768):
    n, d = x.shape
    ntiles = _ceil(_ceil(n, n_cores), 128)
    own = ntiles * 128          # 128-aligned ownership (6272)
    ownp = own
    npad = _ceil(max(n_cores * ownp, n), 512) * 512

    meta_l, cores_l = _prep_conv(lower_indices, lower_values, n, n_cores, own,
                                 ntiles, split)
    meta_u, cores_u = _prep_conv(upper_indices, upper_values, n, n_cores, own,
                                 ntiles, split)

    # combined st-gather indices: per tile, conv-l chunks then conv-u chunks
    offc_l = np.cumsum([0] + meta_l["ch"])
    offc_u = np.cumsum([0] + meta_u["ch"])
    cht_c = meta_l["cht"] + meta_u["cht"]
    # combined xs-gather indices: per tile, [A_l | A_u] and [B_l | B_u]
    offa_l = np.cumsum([0] + meta_l["ncha"])
    offa_u = np.cumsum([0] + meta_u["ncha"])
    offb_l = np.cumsum([0] + meta_l["nchb"])
    offb_u = np.cumsum([0] + meta_u["nchb"])
    fa_c = (sum(meta_l["ncha"]) + sum(meta_u["ncha"])) * 8
    fb_c = max((sum(meta_l["nchb"]) + sum(meta_u["nchb"])) * 8, 8)
    for k in range(n_cores):
        tic = np.zeros((128, cht_c * 8), np.int16)
        iac = np.zeros((128, fa_c), np.int16)
        ibc = np.zeros((128, fb_c), np.int16)
        col = ca = cb = 0
        for t in range(ntiles):
            for cores, offc, offa, offb, meta in (
                    (cores_l, offc_l, offa_l, offb_l, meta_l),
                    (cores_u, offc_u, offa_u, offb_u, meta_u)):
                chv = meta["ch"][t]
                tic[:, col * 8:(col + chv) * 8] = \
                    cores[k]["toffi"][:, offc[t] * 8:(offc[t] + chv) * 8]
                col += chv
            for cores, offa, meta in ((cores_l, offa_l, meta_l),
                                      (cores_u, offa_u, meta_u)):
                nav = meta["ncha"][t]
                iac[:, ca * 8:(ca + nav) * 8] = \
                    cores[k]["idxa"][:, offa[t] * 8:(offa[t] + nav) * 8]
                ca += nav
            for cores, offb, meta in ((cores_l, offb_l, meta_l),
                                      (cores_u, offb_u, meta_u)):
                nbv = meta["nchb"][t]
                if nbv:
                    ibc[:, cb * 8:(cb + nbv) * 8] = \
                        cores[k]["idxb"][:, offb[t] * 8:(offb[t] + nbv) * 8]
                cb += nbv
        cores_l[k]["toffi_c"] = tic
        cores_l[k]["idxa_c"] = iac
        cores_l[k]["idxb_c"] = ibc
        # toff values as rows (partition 0): per tile, l-chunks then u-chunks,
        # chunk-major 128 edges each
        tr = np.zeros((1, cht_c * 128), ml_dtypes.bfloat16)
        col = 0
        for t in range(ntiles):
            for cores, offc, meta in ((cores_l, offc_l, meta_l),
                                      (cores_u, offc_u, meta_u)):
                chv = meta["ch"][t]
                blk = cores[k]["toff"][:, offc[t]:offc[t] + chv]  # [128, chv]
                tr[0, col * 128:(col + chv) * 128] = \
                    blk.T.reshape(-1).astype(ml_dtypes.bfloat16)
                col += chv
        cores_l[k]["tofrow"] = tr
    meta_l["cht_c"] = cht_c
    meta_l["fa_c"] = fa_c
    meta_l["fb_c"] = fb_c

    xf = np.asarray(x, np.float32)
    x_pad = np.zeros((npad, d), np.float32)
    x_pad[:n] = xf
    xs_np = np.zeros((npad, ROW), ml_dtypes.bfloat16)
    xs_np[:, :d] = x_pad.astype(ml_dtypes.bfloat16)
    xt_b = np.ascontiguousarray(x_pad.T).astype(ml_dtypes.bfloat16)  # [d, npad]
    xt2 = np.ascontiguousarray(
        xt_b.reshape(2, 128, npad).transpose(1, 0, 2))  # [128, 2, npad]

    wl = np.asarray(W_lower, np.float32)
    wu = np.asarray(W_upper, np.float32)
    wlin = np.asarray(W_lin, np.float32) * EPS
    al = np.asarray(att_lower, np.float32)
    au = np.asarray(att_upper, np.float32)
    # wa4 cols -> fields: [ss_l, ss_u, st_l, st_u]
    wa4 = np.stack([wl @ al[d:], wu @ au[d:], wl @ al[:d], wu @ au[:d]],
                   axis=1).astype(ml_dtypes.bfloat16)   # [d, 4]
    wa42 = np.ascontiguousarray(wa4.reshape(2, 128, 4).transpose(1, 0, 2))

    def w2(w):
        return np.ascontiguousarray(
            w.astype(ml_dtypes.bfloat16).reshape(2, 128, d).transpose(1, 0, 2))

    iota_np = np.broadcast_to(np.arange(128), (128, 128)).astype(
        ml_dtypes.bfloat16)
    iotap_np = np.broadcast_to(np.arange(128)[:, None, None],
                               (128, 4, 128)).astype(np.float32)

    common = dict(xs_tab=xs_np, xt2=xt2, wa42=wa42,
                  w_l2=w2(wl), w_u2=w2(wu), w_lin2=w2(wlin),
                  iota_in=iota_np, iotap_in=iotap_np)
    in_maps = []
    for k in range(n_cores):
        m = dict(common)
        xo = np.ascontiguousarray(x_pad[k * own:k * own + ownp].T).astype(
            ml_dtypes.bfloat16)
        m["xot2"] = np.ascontiguousarray(
            xo.reshape(2, 128, ownp).transpose(1, 0, 2))
        for cv, cores in (("l", cores_l), ("u", cores_u)):
            cdk = cores[k]
            m[f"toff_{cv}"] = cdk["toff"]
            m[f"val_{cv}"] = cdk["val"]
        m["tofrow"] = cores_l[k]["tofrow"]
        m["idxa_c"] = cores_l[k]["idxa_c"]
        m["idxb_c"] = cores_l[k]["idxb_c"]
        in_maps.append(m)

    dims = dict(n=n, npad=npad, d=d, n_cores=n_cores, own=own, ntiles=ntiles,
                split=split)
    return dims, meta_l, meta_u, in_maps


def build_all(inputs, n_cores=8, split=32768):
    dims, meta_l, meta_u, in_maps = _prepare(**inputs, n_cores=n_cores,
                                             split=split)
    nc = _build_program(dims["n"], dims["npad"], dims["d"], dims["n_cores"],
                        dims["own"], dims["ntiles"], dims["split"],
                        meta_l, meta_u)
    return nc, in_maps, dims


def kernel(**inputs):
    nc, in_maps, dims = build_all(inputs)
    res = run_bass_kernel_spmd(nc, in_maps, list(range(dims["n_cores"])))
    outs = [res.results[k]["out"][:dims["own"]] for k in range(dims["n_cores"])]
    return np.concatenate(outs, axis=0)[:dims["n"]].astype(np.float32)
